# revision 2
# baseline (speedup 1.0000x reference)
import sys

sys.path.insert(0, "/opt/trn_rl_repo")

import math

import numpy as np

import concourse.bass as bass
import concourse.mybir as mybir
import concourse.tile as tile
from concourse import bacc
from concourse.bass_utils import run_bass_kernel_spmd
from concourse.masks import make_identity

F32 = mybir.dt.float32
BF16 = mybir.dt.bfloat16
IDENT = mybir.ActivationFunctionType.Identity
EXPF = mybir.ActivationFunctionType.Exp

B, S, D = 8, 1024, 1024
N_H = 16
REL_K = 16
d_k = D // N_H  # 64
N_CORES = 8
MASKVAL = -1e30

_CACHE = {}
TRACE = False


def build_module():
    nc = bacc.Bacc("TRN2", detect_race_conditions=False, num_swdge_queues=4)

    xT = nc.dram_tensor("xT", [D, S], BF16, kind="ExternalInput")
    Wqk = nc.dram_tensor("Wqk", [D, 2 * D], BF16, kind="ExternalInput")
    Wv = nc.dram_tensor("Wv", [D, D], BF16, kind="ExternalInput")
    Wp = nc.dram_tensor("Wp", [D, D], BF16, kind="ExternalInput")
    bqk = nc.dram_tensor("bqk", [128, 16], F32, kind="ExternalInput")
    bvp = nc.dram_tensor("bvp", [1, D], BF16, kind="ExternalInput")
    bp = nc.dram_tensor("bp", [1, D], BF16, kind="ExternalInput")
    dlut = nc.dram_tensor("dlut", [d_k, 16], BF16, kind="ExternalInput")
    dlv = nc.dram_tensor("dlv", [16, d_k], BF16, kind="ExternalInput")
    selm = nc.dram_tensor("selm", [16, 1024], BF16, kind="ExternalInput")
    zb2init = nc.dram_tensor("zb2init", [128, 160], BF16, kind="ExternalInput")
    zeros64 = nc.dram_tensor("zeros64", [64, 1040], BF16, kind="ExternalInput")
    OUT = nc.dram_tensor("OUT", [S, D], F32, kind="ExternalOutput")

    # DRAM scratch
    zb2r = [nc.dram_tensor(f"zb2r_{r}", [8 * 128, 160], BF16) for r in range(4)]
    zdp = [nc.dram_tensor(f"zdp_{r}", [32, 1040], BF16) for r in range(3)]
    ewd = [nc.dram_tensor(f"ewd_{r}", [128, 1152], BF16) for r in range(2)]
    eshd = [nc.dram_tensor(f"eshd_{r}", [32, 1040], BF16) for r in range(2)]

    with tile.TileContext(nc) as tc:
        with (
            tc.tile_pool(name="persist", bufs=1) as pers,
        ):
            # ---- constants ----
            identF = pers.tile([128, 128], F32)
            make_identity(nc, identF[:])
            identB = pers.tile([128, 128], BF16)
            nc.vector.tensor_copy(identB[:], identF[:])
            dlut_sb = pers.tile([128, 16], BF16)
            nc.sync.dma_start(out=dlut_sb[0:64, :], in_=dlut[:])
            nc.sync.dma_start(out=dlut_sb[64:128, :], in_=dlut[:])
            dlv_sb = pers.tile([128, d_k], BF16)
            nc.sync.dma_start(out=dlv_sb[0:16, :], in_=dlv[:])
            nc.sync.dma_start(out=dlv_sb[64:80, :], in_=dlv[:])
            selm_sb = pers.tile([16, 1024], BF16)
            nc.sync.dma_start(out=selm_sb[:], in_=selm[:])
            bqk_sb = pers.tile([128, 16], F32)
            nc.sync.dma_start(out=bqk_sb[:], in_=bqk[:])
            bvp_sb = pers.tile([1, D], BF16)
            nc.sync.dma_start(out=bvp_sb[:], in_=bvp[:])
            bp_sb = pers.tile([1, D], BF16)
            nc.sync.dma_start(out=bp_sb[:], in_=bp[:])
            ones1 = pers.tile([1, 128], BF16)
            nc.vector.memset(ones1[:], 1.0)
            onescol = pers.tile([128, 16], BF16)
            nc.vector.memset(onescol[:], 1.0)

            # init DRAM scratch: masks + zero regions
            zb2i_sb = pers.tile([128, 160], BF16)
            nc.gpsimd.dma_start(out=zb2i_sb[:], in_=zb2init[:])
            for r in range(4):
                for jt in range(8):
                    dst = bass.AP(tensor=zb2r[r][:].tensor, offset=20480 * jt,
                                  ap=[[160, 128], [1, 160]])
                    nc.gpsimd.dma_start(out=dst, in_=zb2i_sb[:])
            z64 = pers.tile([64, 1040], BF16)
            nc.scalar.dma_start(out=z64[:], in_=zeros64[:])
            for r in range(3):
                nc.scalar.dma_start(out=zdp[r][:], in_=z64[0:32, :])
            for r in range(2):
                nc.scalar.dma_start(out=eshd[r][:], in_=z64[0:32, :])

            # ---- load xT (bf16) ----
            xT_sb = []
            for d in range(8):
                t = pers.tile([128, S], BF16, tag=f"xT{d}")
                eng = nc.sync if d % 2 == 0 else nc.scalar
                eng.dma_start(out=t[:], in_=xT[128 * d:128 * (d + 1), :])
                xT_sb.append(t)

            # ---- v projection -> vhat_sb [128, 16*65] bf16 ----
            vhat_sb = [pers.tile([128, 16 * 65], BF16, name=f"vh{jt}", tag=f"vh{jt}")
                       for jt in range(8)]
            with (
                tc.tile_pool(name="wv", bufs=1) as wvp,
                tc.tile_pool(name="ps_init", bufs=2, space="PSUM") as ps_init,
            ):
                Wv_sb = []
                for d in range(8):
                    t = wvp.tile([128, D], BF16, tag=f"wv{d}")
                    eng = nc.sync if d % 2 == 0 else nc.scalar
                    eng.dma_start(out=t[:], in_=Wv[128 * d:128 * (d + 1), :])
                    Wv_sb.append(t)
                for tt in range(8):
                    vt = vhat_sb[tt]
                    ones_ap = bass.AP(tensor=vt[:].tensor, offset=64,
                                      ap=[[16 * 65, 128], [65, 16]])
                    nc.vector.tensor_copy(ones_ap, onescol[:])
                    for fc in range(2):
                        ps = ps_init.tile([128, 512], F32, tag="pvi")
                        for d in range(8):
                            nc.tensor.matmul(
                                ps[:],
                                xT_sb[d][:, 128 * tt:128 * (tt + 1)],
                                Wv_sb[d][:, 512 * fc:512 * (fc + 1)],
                                start=(d == 0), stop=False,
                            )
                        nc.tensor.matmul(
                            ps[:], ones1[:], bvp_sb[:, 512 * fc:512 * (fc + 1)],
                            start=False, stop=True,
                        )
                        srcA = bass.AP(tensor=ps[:].tensor, offset=ps[:].offset,
                                       ap=[[512, 128], [64, 8], [1, 64]])
                        dst = bass.AP(tensor=vt[:].tensor, offset=65 * 8 * fc,
                                      ap=[[16 * 65, 128], [65, 8], [1, 64]])
                        nc.scalar.copy(dst, srcA)

            pair_sb = [pers.tile([128, S], BF16, name=f"pair{hp}", tag=f"pair{hp}")
                       for hp in range(8)]
            denoms = pers.tile([16, S], BF16)

            # rotating SBUF tiles
            ew_all = [pers.tile([128, 8 * 512], BF16, name=f"ewall{r}",
                                tag=f"ewall{r}") for r in range(2)]
            for r in range(2):
                # jt=7 cols [128,144) of its chunk never written by exp
                nc.vector.memset(ew_all[r][:, 512 * 7 + 128:512 * 7 + 144], 0.0)
            band_all = [pers.tile([128, 8 * 160], BF16, name=f"bandall{r}",
                                  tag=f"bandall{r}") for r in range(4)]
            dpSh4 = [pers.tile([128, 1024], BF16, name=f"dpSh{r}", tag=f"dpSh{r}")
                     for r in range(3)]
            dp_all4 = [pers.tile([128, 1024], BF16, name=f"dpall{r}",
                                 tag=f"dpall{r}") for r in range(3)]
            t1_all4 = [pers.tile([128, 1024], BF16, name=f"t1all{r}",
                                 tag=f"t1all{r}") for r in range(2)]
            esh4 = [pers.tile([128, 1024], BF16, name=f"esh{r}", tag=f"esh{r}")
                    for r in range(2)]
            esk_all = [pers.tile([128, 128], BF16, name=f"eskall{r}",
                                 tag=f"eskall{r}") for r in range(3)]
            dpS_all = [pers.tile([128, 128], BF16, name=f"dpSall{r}",
                                 tag=f"dpSall{r}") for r in range(4)]

            # ---- attention ----
            with (
                tc.tile_pool(name="wqk", bufs=2) as wqkp,
                tc.tile_pool(name="qk", bufs=6) as qkp,
                tc.tile_pool(name="tail", bufs=4) as tailp,
                tc.tile_pool(name="outtp", bufs=2) as outtp,
                tc.tile_pool(name="ps_s", bufs=4, space="PSUM") as ps_s,
                tc.tile_pool(name="ps_out", bufs=1, space="PSUM") as ps_out,
                tc.tile_pool(name="ps_m", bufs=2, space="PSUM") as ps_m,
            ):

                def emit_wload(hp):
                    w_all = wqkp.tile([128, 2048], BF16, name="w_all",
                                      tag="wqk")
                    src = bass.AP(tensor=Wqk[:].tensor, offset=hp * 128 * 2048,
                                  ap=[[2048, 128], [1, 2048]])
                    nc.sync.dma_start(out=w_all[:], in_=src)
                    return w_all

                def emit_qkproj(hp, w_all):
                    """Produce qT/kT [128 feats(2 heads), 1024 tokens]."""
                    out_pair = []
                    for sec in range(2):
                        ft = hp if sec == 0 else 8 + hp
                        dstt = qkp.tile([128, S], BF16, tag=f"qk{sec}")
                        for tch in range(2):
                            ps = ps_s.tile([128, 512], F32, tag="pss")
                            for d in range(8):
                                nc.tensor.matmul(
                                    ps[:],
                                    w_all[:, 1024 * sec + 128 * d:
                                          1024 * sec + 128 * (d + 1)],
                                    xT_sb[d][:, 512 * tch:512 * (tch + 1)],
                                    start=(d == 0), stop=(d == 7),
                                )
                            nc.scalar.activation(
                                dstt[:, 512 * tch:512 * (tch + 1)], ps[:],
                                IDENT, bias=bqk_sb[:, ft:ft + 1], scale=1.0)
                        out_pair.append(dstt)
                    return out_pair

                qk_tiles = {}  # hp -> (qT, kT)
                relv_state = {}  # hp -> open pso2 tile

                def emit_dp(hp):
                    """dp matmuls + zdp/dpSh shear bounce for pair hp."""
                    pr = hp % 3
                    qT_p = qk_tiles[hp][0]
                    psA = ps_s.tile([128, 512], F32, name="psA", tag="pss")
                    psB = ps_s.tile([128, 512], F32, name="psB", tag="pss")
                    for k in range(2):
                        pok = 64 * k
                        nc.tensor.matmul(
                            psA[64 * k:64 * k + 16, :],
                            dlut_sb[pok:pok + 64, :],
                            qT_p[pok:pok + 64, 0:512],
                            start=True, stop=True, skip_group_check=True)
                        nc.tensor.matmul(
                            psB[64 * k:64 * k + 16, :],
                            dlut_sb[pok:pok + 64, :],
                            qT_p[pok:pok + 64, 512:1024],
                            start=True, stop=True, skip_group_check=True)
                    for k in range(2):
                        nc.vector.tensor_copy(
                            dp_all4[pr][64 * k:64 * k + 16, 0:512],
                            psA[64 * k:64 * k + 16, :])
                        nc.vector.tensor_copy(
                            dp_all4[pr][64 * k:64 * k + 16, 512:1024],
                            psB[64 * k:64 * k + 16, :])
                    for k in range(2):
                        dstz = bass.AP(tensor=zdp[pr][:].tensor,
                                       offset=16640 * k,
                                       ap=[[1040, 16], [1, 1024]])
                        nc.gpsimd.dma_start(out=dstz,
                                            in_=dp_all4[pr][64 * k:64 * k + 16,
                                                            :])
                        srcSh = bass.AP(tensor=zdp[pr][:].tensor,
                                        offset=16640 * k,
                                        ap=[[1041, 16], [1, 1024]])
                        nc.gpsimd.dma_start(
                            out=dpSh4[pr][64 * k:64 * k + 16, :], in_=srcSh)

                def emit_s1(h):
                    """band pipeline for head h: transposes -> diag write ->
                    band+mask rect read."""
                    hp, hh = h // 2, h % 2
                    pr, hr = hp % 3, h % 4
                    psd = ps_m.tile([128, 512], BF16, name="psd", tag="trp")
                    for jt in range(8):
                        nc.tensor.transpose(
                            psd[:, 16 * jt:16 * (jt + 1)],
                            dpSh4[pr][64 * hh:64 * hh + 16,
                                      128 * jt:128 * (jt + 1)],
                            identB[64 * hh:64 * hh + 16,
                                   64 * hh:64 * hh + 16])
                    nc.vector.tensor_copy(dpS_all[hr][:], psd[:, 0:128])
                    dstW = bass.AP(tensor=zb2r[hr][:].tensor, offset=0,
                                   ap=[[161, 128], [20480, 8], [1, 16]])
                    nc.gpsimd.dma_start(out=dstW, in_=dpS_all[hr][:])
                    srcB = bass.AP(tensor=zb2r[hr][:].tensor, offset=0,
                                   ap=[[160, 128], [20480, 8], [1, 160]])
                    nc.gpsimd.dma_start(out=band_all[hr][:], in_=srcB)

                def emit_s2(h):
                    """scores + band add + exp + attn@v + evict + ew/esk DMAs."""
                    hp, hh = h // 2, h % 2
                    po = 64 * hh
                    hr = h % 4
                    ewr = h % 2
                    qT, kT = qk_tiles[hp]
                    pso = ps_out.tile([65, 1024], F32, name="pso", tag="pso")
                    ew = ew_all[ewr]
                    seg_jobs = []

                    def flush_av(jx):
                        lhs, segs = seg_jobs[jx]
                        for si, (a, b2, mt, off) in enumerate(segs):
                            nc.tensor.matmul(
                                pso[:, a:b2], lhs,
                                mt[:, a + off:b2 + off],
                                start=(jx == 0),
                                stop=(jx == 7 and si == len(segs) - 1),
                                skip_group_check=True)

                    for jt in range(8):
                        j0 = 128 * jt
                        wdiag = min(512, S - j0)
                        win = min(144, wdiag)
                        pss = ps_s.tile([128, 512], F32, name="pss", tag="pss")
                        nc.tensor.matmul(pss[:, 0:wdiag],
                                         kT[po:po + 64, j0:j0 + 128],
                                         qT[po:po + 64, j0:j0 + wdiag],
                                         start=True, stop=False,
                                         skip_group_check=True)
                        nc.tensor.matmul(
                            pss[:, 0:win], identB[:],
                            band_all[hr][:, 160 * jt:160 * jt + win],
                            start=False, stop=True, skip_group_check=True)
                        nc.scalar.activation(
                            ew[:, 512 * jt:512 * jt + wdiag],
                            pss[:, 0:wdiag], EXPF)
                        tail = None
                        if S - j0 > 512:
                            w2 = S - j0 - 512
                            pss2 = ps_s.tile([128, 512], F32, name="pss2",
                                             tag="pss")
                            nc.tensor.matmul(pss2[:, 0:w2],
                                             kT[po:po + 64, j0:j0 + 128],
                                             qT[po:po + 64, j0 + 512:S],
                                             start=True, stop=True)
                            tail = tailp.tile([128, 512], BF16, name="tail",
                                              tag="tail")
                            nc.scalar.activation(tail[:, 0:w2], pss2[:, 0:w2],
                                                 EXPF)
                        lhs = vhat_sb[jt][:, 65 * h:65 * h + 65]
                        segs = []
                        if j0 < 512:
                            segs.append((j0, 512, ew, 512 * jt - j0))
                            if j0 + wdiag > 512:
                                segs.append((512, j0 + wdiag, ew,
                                             512 * jt - j0))
                        else:
                            segs.append((j0, j0 + wdiag, ew, 512 * jt - j0))
                        if tail is not None:
                            segs.append((j0 + 512, S, tail, -(j0 + 512)))
                        seg_jobs.append((lhs, segs))
                        if jt >= 3:
                            flush_av(jt - 3)
                    flush_av(5)
                    flush_av(6)
                    flush_av(7)
                    # evict attn@v + denom
                    outT = outtp.tile([65, 1024], BF16, name="outT", tag="outT")
                    nc.vector.tensor_copy(outT[:], pso[:])
                    nc.sync.dma_start(out=pair_sb[hp][po:po + 64, :],
                                      in_=outT[0:64, :])
                    nc.sync.dma_start(out=denoms[h:h + 1, :], in_=outT[64:65, :])
                    # ew band write + esk diag read (SP queue, ordered)
                    dstE = bass.AP(tensor=ewd[ewr][:].tensor, offset=0,
                                   ap=[[1152, 128], [144, 8], [1, 144]])
                    srcE = bass.AP(tensor=ew[:].tensor, offset=ew[:].offset,
                                   ap=[[8 * 512, 128], [512, 8], [1, 144]])
                    nc.scalar.dma_start(out=dstE, in_=srcE)
                    srcK = bass.AP(tensor=ewd[ewr][:].tensor, offset=0,
                                   ap=[[1153, 128], [144, 8], [1, 16]])
                    nc.scalar.dma_start(out=esk_all[h % 3][:], in_=srcK)

                def emit_s3(h):
                    """esk transposes + t1 assembly for head h; at pair end,
                    t1 diag write + esh read."""
                    hp, hh = h // 2, h % 2
                    pr = hp % 2
                    eskT = ps_m.tile([128, 512], BF16, name="eskT", tag="trp")
                    eskT2 = ps_m.tile([128, 512], BF16, name="eskT2", tag="trp")
                    for jt in range(8):
                        dst_ps = eskT if jt < 4 else eskT2
                        nc.tensor.transpose(
                            dst_ps[64 * hh:64 * hh + 16,
                                   128 * (jt % 4):128 * (jt % 4) + 128],
                            esk_all[h % 3][:, 16 * jt:16 * (jt + 1)],
                            identB[:])
                    nc.vector.tensor_copy(
                        t1_all4[pr][64 * hh:64 * hh + 16, 0:512],
                        eskT[64 * hh:64 * hh + 16, :])
                    nc.vector.tensor_copy(
                        t1_all4[pr][64 * hh:64 * hh + 16, 512:1024],
                        eskT2[64 * hh:64 * hh + 16, :])
                    if hh == 1:
                        for k in range(2):
                            dstT = bass.AP(tensor=eshd[pr][:].tensor,
                                           offset=16640 * k,
                                           ap=[[1041, 16], [1, 1024]])
                            nc.sync.dma_start(
                                out=dstT,
                                in_=t1_all4[pr][64 * k:64 * k + 16, :])
                            srcS2 = bass.AP(tensor=eshd[pr][:].tensor,
                                            offset=16640 * k,
                                            ap=[[1040, 16], [1, 1024]])
                            nc.sync.dma_start(
                                out=esh4[pr][64 * k:64 * k + 16, :],
                                in_=srcS2)

                def emit_relv(h):
                    """rel-v matmuls for head h, folded into pair_sb."""
                    hh2 = h % 2
                    hp = h // 2
                    pr = hp % 2
                    po2 = 64 * hh2
                    for cc in range(2):
                        pr2 = ps_m.tile([128, 512], F32, name="pr2", tag="trp")
                        nc.tensor.matmul(
                            pr2[po2:po2 + 64, :],
                            dlv_sb[po2:po2 + 16, :],
                            esh4[pr][po2:po2 + 16, 512 * cc:512 * (cc + 1)],
                            start=True, stop=True, skip_group_check=True,
                        )
                        nc.vector.tensor_add(
                            pair_sb[hp][po2:po2 + 64, 512 * cc:512 * (cc + 1)],
                            pair_sb[hp][po2:po2 + 64, 512 * cc:512 * (cc + 1)],
                            pr2[po2:po2 + 64, :])

                # software-pipelined schedule
                w0 = emit_wload(0)
                w1 = emit_wload(1)
                qk_tiles[0] = emit_qkproj(0, w0)
                qk_tiles[1] = emit_qkproj(1, w1)
                emit_dp(0)
                emit_dp(1)
                emit_s1(0)
                emit_s1(1)
                w_next = emit_wload(2)
                relv_q = []
                for h in range(16):
                    hp, hh = h // 2, h % 2
                    if hh == 1 and hp + 2 < 8:
                        emit_dp(hp + 2)
                    if h + 2 < 16:
                        emit_s1(h + 2)
                    if h >= 2:
                        emit_s3(h - 2)
                        if (h - 2) % 2 == 1:
                            qq = (h - 2) // 2
                            relv_q += [2 * qq, 2 * qq + 1]
                    if relv_q:
                        emit_relv(relv_q.pop(0))
                    emit_s2(h)
                    if hh == 0 and hp + 2 < 8:
                        qk_tiles[hp + 2] = emit_qkproj(hp + 2, w_next)
                        if hp + 3 < 8:
                            w_next = emit_wload(hp + 3)
                emit_s3(14)
                emit_s3(15)
                relv_q += [14, 15]
                while relv_q:
                    emit_relv(relv_q.pop(0))

            # ---- normalize ----
            recip = pers.tile([16, S], BF16)
            with nc.allow_low_precision(reason="bf16 softmax normalization"):
                nc.vector.reciprocal(recip[:], denoms[:])
            with tc.tile_pool(name="ps_n", bufs=2, space="PSUM") as ps_n:
                for hp in range(8):
                    psb = ps_n.tile([128, 1024], F32, tag="psb")
                    for c in range(2):
                        nc.tensor.matmul(psb[:, 512 * c:512 * (c + 1)],
                                         selm_sb[:, 128 * hp:128 * (hp + 1)],
                                         recip[:, 512 * c:512 * (c + 1)],
                                         start=True, stop=True)
                    nc.vector.tensor_mul(pair_sb[hp][:], pair_sb[hp][:], psb[:])

            # ---- final projection ----
            with (
                tc.tile_pool(name="wp", bufs=1) as wpp,
                tc.tile_pool(name="ps_p", bufs=2, space="PSUM") as ps_p,
                tc.tile_pool(name="outp", bufs=2) as outp,
            ):
                Wp_sb = []
                for d in range(8):
                    t = wpp.tile([128, D], BF16, tag=f"wp{d}")
                    nc.sync.dma_start(out=t[:], in_=Wp[128 * d:128 * (d + 1), :])
                    Wp_sb.append(t)
                for tt in range(8):
                    ps = ps_p.tile([128, 1024], F32, tag="psp")
                    for fc in range(2):
                        for d in range(8):
                            nc.tensor.matmul(
                                ps[:, 512 * fc:512 * (fc + 1)],
                                pair_sb[d][:, 128 * tt:128 * (tt + 1)],
                                Wp_sb[d][:, 512 * fc:512 * (fc + 1)],
                                start=(d == 0), stop=False,
                            )
                        nc.tensor.matmul(
                            ps[:, 512 * fc:512 * (fc + 1)],
                            ones1[:], bp_sb[:, 512 * fc:512 * (fc + 1)],
                            start=False, stop=True,
                        )
                    ot = outp.tile([128, 1024], F32, tag="ot")
                    nc.vector.tensor_copy(ot[:], ps[:])
                    nc.sync.dma_start(out=OUT[128 * tt:128 * (tt + 1), :],
                                      in_=ot[:])

    nc.compile()
    return nc


def _to_bf16(a):
    import ml_dtypes
    return np.asarray(a, np.float32).astype(ml_dtypes.bfloat16)


def _host_prep(W_attn, b_attn, W_proj, b_proj, lut_k, lut_v):
    scale = 1.0 / math.sqrt(d_k)
    Wq = W_attn[:, :D]
    Wk = W_attn[:, D:2 * D] * scale
    # packed per-pair layout: [hp, p, sec*1024 + d*128 + c]
    Wq4 = Wq.reshape(8, 128, 8, 128).transpose(2, 1, 0, 3).reshape(8, 128, 1024)
    Wk4 = Wk.reshape(8, 128, 8, 128).transpose(2, 1, 0, 3).reshape(8, 128, 1024)
    Wqk_h = np.concatenate([Wq4, Wk4], axis=2).reshape(1024, 2048)
    bq = b_attn[:D]
    bk = b_attn[D:2 * D] * scale
    bqk_h = np.stack([np.concatenate([bq, bk])[128 * ft:128 * (ft + 1)]
                      for ft in range(16)], axis=1).astype(np.float32)
    bvp_h = (b_attn[2 * D:3 * D] + np.tile(lut_v[0], N_H)).reshape(1, D)
    dlut_h = np.stack([(lut_k[16 - u] - lut_k[0]) * scale for u in range(16)],
                      axis=1)
    dlv_h = np.stack([lut_v[16 - u] - lut_v[0] for u in range(16)], axis=0)
    selm_h = np.zeros((16, 1024), np.float32)
    for hp in range(8):
        for p in range(128):
            selm_h[2 * hp + p // 64, 128 * hp + p] = 1.0
    zb2_h = np.where(np.arange(160)[None, :] < np.arange(128)[:, None],
                     np.float32(MASKVAL), np.float32(0.0)).astype(np.float32)
    return {
        "Wqk": _to_bf16(Wqk_h),
        "Wv": _to_bf16(W_attn[:, 2 * D:3 * D]),
        "Wp": _to_bf16(W_proj),
        "bqk": bqk_h,
        "bvp": _to_bf16(bvp_h),
        "bp": _to_bf16(np.asarray(b_proj).reshape(1, D)),
        "dlut": _to_bf16(dlut_h),
        "dlv": _to_bf16(dlv_h),
        "selm": _to_bf16(selm_h),
        "zb2init": _to_bf16(zb2_h),
        "zeros64": _to_bf16(np.zeros((64, 1040), np.float32)),
    }


def kernel(x, W_attn, b_attn, W_proj, b_proj, lut_k, lut_v):
    x = np.asarray(x, np.float32)
    shared = _host_prep(np.asarray(W_attn, np.float32),
                        np.asarray(b_attn, np.float32),
                        np.asarray(W_proj, np.float32),
                        np.asarray(b_proj, np.float32),
                        np.asarray(lut_k, np.float32),
                        np.asarray(lut_v, np.float32))
    if "nc" not in _CACHE:
        _CACHE["nc"] = build_module()
    nc = _CACHE["nc"]
    in_maps = []
    for b in range(N_CORES):
        m = dict(shared)
        m["xT"] = _to_bf16(np.ascontiguousarray(x[b].T))
        in_maps.append(m)
    res = run_bass_kernel_spmd(nc, in_maps, list(range(N_CORES)), trace=TRACE)
    _CACHE["last_result"] = res
    out = np.stack([res.results[b]["OUT"] for b in range(N_CORES)], axis=0)
    return out.astype(np.float32)


# revision 3
# speedup vs baseline: 1.0258x; 1.0258x over previous
import sys

sys.path.insert(0, "/opt/trn_rl_repo")

import math

import numpy as np

import concourse.bass as bass
import concourse.mybir as mybir
import concourse.tile as tile
from concourse import bacc
from concourse.bass_utils import run_bass_kernel_spmd
from concourse.masks import make_identity

F32 = mybir.dt.float32
BF16 = mybir.dt.bfloat16
IDENT = mybir.ActivationFunctionType.Identity
EXPF = mybir.ActivationFunctionType.Exp

B, S, D = 8, 1024, 1024
N_H = 16
REL_K = 16
d_k = D // N_H  # 64
N_CORES = 8
MASKVAL = -1e30

_CACHE = {}
TRACE = False


def build_module():
    nc = bacc.Bacc("TRN2", detect_race_conditions=False, num_swdge_queues=4)

    xT = nc.dram_tensor("xT", [D, S], BF16, kind="ExternalInput")
    Wqk = nc.dram_tensor("Wqk", [D, 2 * D], BF16, kind="ExternalInput")
    Wv = nc.dram_tensor("Wv", [D, D], BF16, kind="ExternalInput")
    Wp = nc.dram_tensor("Wp", [D, D], BF16, kind="ExternalInput")
    bqk = nc.dram_tensor("bqk", [128, 16], F32, kind="ExternalInput")
    bvp = nc.dram_tensor("bvp", [1, D], BF16, kind="ExternalInput")
    bp = nc.dram_tensor("bp", [1, D], BF16, kind="ExternalInput")
    dlut = nc.dram_tensor("dlut", [d_k, 16], BF16, kind="ExternalInput")
    dlv = nc.dram_tensor("dlv", [16, d_k], BF16, kind="ExternalInput")
    selm2 = nc.dram_tensor("selm2", [2, 128], BF16, kind="ExternalInput")
    zb2init = nc.dram_tensor("zb2init", [128, 160], BF16, kind="ExternalInput")
    zeros64 = nc.dram_tensor("zeros64", [64, 1040], BF16, kind="ExternalInput")
    OUT = nc.dram_tensor("OUT", [S, D], F32, kind="ExternalOutput")

    # DRAM scratch
    zb2r = [nc.dram_tensor(f"zb2r_{r}", [8 * 128, 160], BF16) for r in range(4)]
    zdp = [nc.dram_tensor(f"zdp_{r}", [32, 1040], BF16) for r in range(3)]
    ewd = [nc.dram_tensor(f"ewd_{r}", [128, 1152], BF16) for r in range(2)]
    eshd = [nc.dram_tensor(f"eshd_{r}", [32, 1040], BF16) for r in range(2)]

    with tile.TileContext(nc) as tc:
        with (
            tc.tile_pool(name="persist", bufs=1) as pers,
        ):
            # ---- load xT first (longest pole for vproj) ----
            xT_sb = []
            for d in range(8):
                t = pers.tile([128, S], BF16, name="xTl", tag=f"xT{d}")
                eng = nc.sync if d % 2 == 0 else nc.scalar
                eng.dma_start(out=t[:], in_=xT[128 * d:128 * (d + 1), :])
                xT_sb.append(t)

            # ---- constants ----
            identF = pers.tile([128, 128], F32)
            make_identity(nc, identF[:])
            identB = pers.tile([128, 128], BF16)
            nc.vector.tensor_copy(identB[:], identF[:])
            dlut_sb = pers.tile([128, 16], BF16)
            nc.sync.dma_start(out=dlut_sb[0:64, :], in_=dlut[:])
            nc.sync.dma_start(out=dlut_sb[64:128, :], in_=dlut[:])
            dlv_sb = pers.tile([128, d_k], BF16)
            nc.sync.dma_start(out=dlv_sb[0:16, :], in_=dlv[:])
            nc.sync.dma_start(out=dlv_sb[64:80, :], in_=dlv[:])
            selm2_sb = pers.tile([2, 128], BF16)
            nc.sync.dma_start(out=selm2_sb[:], in_=selm2[:])
            bqk_sb = pers.tile([128, 16], F32)
            nc.sync.dma_start(out=bqk_sb[:], in_=bqk[:])
            bvp_sb = pers.tile([1, D], BF16)
            nc.sync.dma_start(out=bvp_sb[:], in_=bvp[:])
            bp_sb = pers.tile([1, D], BF16)
            nc.sync.dma_start(out=bp_sb[:], in_=bp[:])
            ones1 = pers.tile([1, 128], BF16)
            nc.vector.memset(ones1[:], 1.0)
            onescol = pers.tile([128, 16], BF16)
            nc.vector.memset(onescol[:], 1.0)

            # init DRAM scratch: masks + zero regions
            zb2i_sb = pers.tile([128, 160], BF16)
            nc.gpsimd.dma_start(out=zb2i_sb[:], in_=zb2init[:])
            for r in range(4):
                for jt in range(8):
                    dst = bass.AP(tensor=zb2r[r][:].tensor, offset=20480 * jt,
                                  ap=[[160, 128], [1, 160]])
                    nc.gpsimd.dma_start(out=dst, in_=zb2i_sb[:])
            z64 = pers.tile([64, 1040], BF16)
            nc.scalar.dma_start(out=z64[:], in_=zeros64[:])
            for r in range(3):
                nc.scalar.dma_start(out=zdp[r][:], in_=z64[0:32, :])
            for r in range(2):
                nc.scalar.dma_start(out=eshd[r][:], in_=z64[0:32, :])

            # ---- v projection -> vhat_sb [128, 16*65] bf16 ----
            vhat_sb = [pers.tile([128, 16 * 65], BF16, name=f"vh{jt}", tag=f"vh{jt}")
                       for jt in range(8)]
            with (
                tc.tile_pool(name="wv", bufs=1) as wvp,
                tc.tile_pool(name="ps_init", bufs=2, space="PSUM") as ps_init,
            ):
                Wv_sb = []
                for d in range(8):
                    t = wvp.tile([128, D], BF16, tag=f"wv{d}")
                    eng = nc.sync if d % 2 == 0 else nc.scalar
                    eng.dma_start(out=t[:], in_=Wv[128 * d:128 * (d + 1), :])
                    Wv_sb.append(t)
                for tt in range(8):
                    vt = vhat_sb[tt]
                    ones_ap = bass.AP(tensor=vt[:].tensor, offset=64,
                                      ap=[[16 * 65, 128], [65, 16]])
                    nc.vector.tensor_copy(ones_ap, onescol[:])
                    for fc in range(2):
                        ps = ps_init.tile([128, 512], F32, tag="pvi")
                        for d in range(8):
                            nc.tensor.matmul(
                                ps[:],
                                xT_sb[d][:, 128 * tt:128 * (tt + 1)],
                                Wv_sb[d][:, 512 * fc:512 * (fc + 1)],
                                start=(d == 0), stop=False,
                            )
                        nc.tensor.matmul(
                            ps[:], ones1[:], bvp_sb[:, 512 * fc:512 * (fc + 1)],
                            start=False, stop=True,
                        )
                        srcA = bass.AP(tensor=ps[:].tensor, offset=ps[:].offset,
                                       ap=[[512, 128], [64, 8], [1, 64]])
                        dst = bass.AP(tensor=vt[:].tensor, offset=65 * 8 * fc,
                                      ap=[[16 * 65, 128], [65, 8], [1, 64]])
                        nc.scalar.copy(dst, srcA)

            pair_sb = [pers.tile([128, S], BF16, name=f"pair{hp}", tag=f"pair{hp}")
                       for hp in range(8)]
            recd = [pers.tile([2, S], BF16, name=f"recd{hp}",
                               tag=f"recd{hp}") for hp in range(8)]

            Wp_sb = [pers.tile([128, D], BF16, name="wpt", tag=f"wp{d}")
                     for d in range(8)]

            # rotating SBUF tiles
            ew_all = [pers.tile([128, 8 * 512], BF16, name=f"ewall{r}",
                                tag=f"ewall{r}") for r in range(2)]
            for r in range(2):
                # jt=7 cols [128,144) of its chunk never written by exp
                nc.vector.memset(ew_all[r][:, 512 * 7 + 128:512 * 7 + 144], 0.0)
            band_all = [pers.tile([128, 8 * 160], BF16, name=f"bandall{r}",
                                  tag=f"bandall{r}") for r in range(4)]
            dpSh4 = [pers.tile([128, 1024], BF16, name=f"dpSh{r}", tag=f"dpSh{r}")
                     for r in range(3)]
            dp_all4 = [pers.tile([128, 1024], BF16, name=f"dpall{r}",
                                 tag=f"dpall{r}") for r in range(3)]
            t1_all4 = [pers.tile([128, 1024], BF16, name=f"t1all{r}",
                                 tag=f"t1all{r}") for r in range(2)]
            esh4 = [pers.tile([128, 1024], BF16, name=f"esh{r}", tag=f"esh{r}")
                    for r in range(2)]
            esk_all = [pers.tile([128, 128], BF16, name=f"eskall{r}",
                                 tag=f"eskall{r}") for r in range(3)]
            dpS_all = [pers.tile([128, 128], BF16, name=f"dpSall{r}",
                                 tag=f"dpSall{r}") for r in range(4)]

            # ---- attention ----
            with (
                tc.tile_pool(name="wqk", bufs=2) as wqkp,
                tc.tile_pool(name="qk", bufs=6) as qkp,
                tc.tile_pool(name="tail", bufs=4) as tailp,
                tc.tile_pool(name="outtp", bufs=2) as outtp,
                tc.tile_pool(name="ps_s", bufs=4, space="PSUM") as ps_s,
                tc.tile_pool(name="ps_out", bufs=1, space="PSUM") as ps_out,
                tc.tile_pool(name="ps_m", bufs=2, space="PSUM") as ps_m,
            ):

                def emit_wload(hp):
                    w_all = wqkp.tile([128, 2048], BF16, name="w_all",
                                      tag="wqk")
                    src = bass.AP(tensor=Wqk[:].tensor, offset=hp * 128 * 2048,
                                  ap=[[2048, 128], [1, 2048]])
                    nc.sync.dma_start(out=w_all[:], in_=src)
                    return w_all

                def emit_qkproj(hp, w_all):
                    """Produce qT/kT [128 feats(2 heads), 1024 tokens]."""
                    out_pair = []
                    for sec in range(2):
                        ft = hp if sec == 0 else 8 + hp
                        dstt = qkp.tile([128, S], BF16, tag=f"qk{sec}")
                        for tch in range(2):
                            ps = ps_s.tile([128, 512], F32, tag="pss")
                            for d in range(8):
                                nc.tensor.matmul(
                                    ps[:],
                                    w_all[:, 1024 * sec + 128 * d:
                                          1024 * sec + 128 * (d + 1)],
                                    xT_sb[d][:, 512 * tch:512 * (tch + 1)],
                                    start=(d == 0), stop=(d == 7),
                                )
                            nc.scalar.activation(
                                dstt[:, 512 * tch:512 * (tch + 1)], ps[:],
                                IDENT, bias=bqk_sb[:, ft:ft + 1], scale=1.0)
                        out_pair.append(dstt)
                    return out_pair

                qk_tiles = {}  # hp -> (qT, kT)
                relv_state = {}  # hp -> open pso2 tile

                def emit_dp(hp):
                    """dp matmuls + zdp/dpSh shear bounce for pair hp."""
                    pr = hp % 3
                    qT_p = qk_tiles[hp][0]
                    psA = ps_s.tile([128, 512], F32, name="psA", tag="pss")
                    psB = ps_s.tile([128, 512], F32, name="psB", tag="pss")
                    for k in range(2):
                        pok = 64 * k
                        nc.tensor.matmul(
                            psA[64 * k:64 * k + 16, :],
                            dlut_sb[pok:pok + 64, :],
                            qT_p[pok:pok + 64, 0:512],
                            start=True, stop=True, skip_group_check=True)
                        nc.tensor.matmul(
                            psB[64 * k:64 * k + 16, :],
                            dlut_sb[pok:pok + 64, :],
                            qT_p[pok:pok + 64, 512:1024],
                            start=True, stop=True, skip_group_check=True)
                    for k in range(2):
                        nc.vector.tensor_copy(
                            dp_all4[pr][64 * k:64 * k + 16, 0:512],
                            psA[64 * k:64 * k + 16, :])
                        nc.vector.tensor_copy(
                            dp_all4[pr][64 * k:64 * k + 16, 512:1024],
                            psB[64 * k:64 * k + 16, :])
                    for k in range(2):
                        dstz = bass.AP(tensor=zdp[pr][:].tensor,
                                       offset=16640 * k,
                                       ap=[[1040, 16], [1, 1024]])
                        nc.gpsimd.dma_start(out=dstz,
                                            in_=dp_all4[pr][64 * k:64 * k + 16,
                                                            :])
                        srcSh = bass.AP(tensor=zdp[pr][:].tensor,
                                        offset=16640 * k,
                                        ap=[[1041, 16], [1, 1024]])
                        nc.gpsimd.dma_start(
                            out=dpSh4[pr][64 * k:64 * k + 16, :], in_=srcSh)

                def emit_s1(h):
                    """band pipeline for head h: transposes -> diag write ->
                    band+mask rect read."""
                    hp, hh = h // 2, h % 2
                    pr, hr = hp % 3, h % 4
                    psd = ps_m.tile([128, 512], BF16, name="psd", tag="trp")
                    for jt in range(8):
                        nc.tensor.transpose(
                            psd[:, 16 * jt:16 * (jt + 1)],
                            dpSh4[pr][64 * hh:64 * hh + 16,
                                      128 * jt:128 * (jt + 1)],
                            identB[64 * hh:64 * hh + 16,
                                   64 * hh:64 * hh + 16])
                    nc.vector.tensor_copy(dpS_all[hr][:], psd[:, 0:128])
                    dstW = bass.AP(tensor=zb2r[hr][:].tensor, offset=0,
                                   ap=[[161, 128], [20480, 8], [1, 16]])
                    nc.gpsimd.dma_start(out=dstW, in_=dpS_all[hr][:])
                    srcB = bass.AP(tensor=zb2r[hr][:].tensor, offset=0,
                                   ap=[[160, 128], [20480, 8], [1, 160]])
                    nc.gpsimd.dma_start(out=band_all[hr][:], in_=srcB)

                def emit_s2(h):
                    """scores + band add + exp + attn@v + evict + ew/esk DMAs."""
                    hp, hh = h // 2, h % 2
                    po = 64 * hh
                    hr = h % 4
                    ewr = h % 2
                    qT, kT = qk_tiles[hp]
                    pso = ps_out.tile([65, 1024], F32, name="pso", tag="pso")
                    ew = ew_all[ewr]
                    seg_jobs = []

                    def flush_av(jx):
                        lhs, segs = seg_jobs[jx]
                        for si, (a, b2, mt, off) in enumerate(segs):
                            nc.tensor.matmul(
                                pso[:, a:b2], lhs,
                                mt[:, a + off:b2 + off],
                                start=(jx == 0),
                                stop=(jx == 7 and si == len(segs) - 1),
                                skip_group_check=True)

                    for jt in range(8):
                        j0 = 128 * jt
                        wdiag = min(512, S - j0)
                        win = min(144, wdiag)
                        pss = ps_s.tile([128, 512], F32, name="pss", tag="pss")
                        nc.tensor.matmul(pss[:, 0:wdiag],
                                         kT[po:po + 64, j0:j0 + 128],
                                         qT[po:po + 64, j0:j0 + wdiag],
                                         start=True, stop=False,
                                         skip_group_check=True)
                        nc.tensor.matmul(
                            pss[:, 0:win], identB[:],
                            band_all[hr][:, 160 * jt:160 * jt + win],
                            start=False, stop=True, skip_group_check=True)
                        nc.scalar.activation(
                            ew[:, 512 * jt:512 * jt + wdiag],
                            pss[:, 0:wdiag], EXPF)
                        tail = None
                        if S - j0 > 512:
                            w2 = S - j0 - 512
                            pss2 = ps_s.tile([128, 512], F32, name="pss2",
                                             tag="pss")
                            nc.tensor.matmul(pss2[:, 0:w2],
                                             kT[po:po + 64, j0:j0 + 128],
                                             qT[po:po + 64, j0 + 512:S],
                                             start=True, stop=True)
                            tail = tailp.tile([128, 512], BF16, name="tail",
                                              tag="tail")
                            nc.scalar.activation(tail[:, 0:w2], pss2[:, 0:w2],
                                                 EXPF)
                        lhs = vhat_sb[jt][:, 65 * h:65 * h + 65]
                        segs = []
                        if j0 < 512:
                            segs.append((j0, 512, ew, 512 * jt - j0))
                            if j0 + wdiag > 512:
                                segs.append((512, j0 + wdiag, ew,
                                             512 * jt - j0))
                        else:
                            segs.append((j0, j0 + wdiag, ew, 512 * jt - j0))
                        if tail is not None:
                            segs.append((j0 + 512, S, tail, -(j0 + 512)))
                        seg_jobs.append((lhs, segs))
                        if jt >= 3:
                            flush_av(jt - 3)
                    flush_av(5)
                    flush_av(6)
                    flush_av(7)
                    # evict attn@v + denom
                    outT = outtp.tile([65, 1024], BF16, name="outT", tag="outT")
                    nc.vector.tensor_copy(outT[:], pso[:])
                    nc.sync.dma_start(out=pair_sb[hp][po:po + 64, :],
                                      in_=outT[0:64, :])
                    nc.sync.dma_start(out=recd[hp][hh:hh + 1, :],
                                      in_=outT[64:65, :])
                    # ew band write + esk diag read (SP queue, ordered)
                    dstE = bass.AP(tensor=ewd[ewr][:].tensor, offset=0,
                                   ap=[[1152, 128], [144, 8], [1, 144]])
                    srcE = bass.AP(tensor=ew[:].tensor, offset=ew[:].offset,
                                   ap=[[8 * 512, 128], [512, 8], [1, 144]])
                    nc.scalar.dma_start(out=dstE, in_=srcE)
                    srcK = bass.AP(tensor=ewd[ewr][:].tensor, offset=0,
                                   ap=[[1153, 128], [144, 8], [1, 16]])
                    nc.scalar.dma_start(out=esk_all[h % 3][:], in_=srcK)

                def emit_s3(h):
                    """esk transposes + t1 assembly for head h; at pair end,
                    t1 diag write + esh read."""
                    hp, hh = h // 2, h % 2
                    pr = hp % 2
                    eskT = ps_m.tile([128, 512], BF16, name="eskT", tag="trp")
                    eskT2 = ps_m.tile([128, 512], BF16, name="eskT2", tag="trp")
                    for jt in range(8):
                        dst_ps = eskT if jt < 4 else eskT2
                        nc.tensor.transpose(
                            dst_ps[64 * hh:64 * hh + 16,
                                   128 * (jt % 4):128 * (jt % 4) + 128],
                            esk_all[h % 3][:, 16 * jt:16 * (jt + 1)],
                            identB[:])
                    nc.vector.tensor_copy(
                        t1_all4[pr][64 * hh:64 * hh + 16, 0:512],
                        eskT[64 * hh:64 * hh + 16, :])
                    nc.vector.tensor_copy(
                        t1_all4[pr][64 * hh:64 * hh + 16, 512:1024],
                        eskT2[64 * hh:64 * hh + 16, :])
                    if hh == 1:
                        for k in range(2):
                            dstT = bass.AP(tensor=eshd[pr][:].tensor,
                                           offset=16640 * k,
                                           ap=[[1041, 16], [1, 1024]])
                            nc.sync.dma_start(
                                out=dstT,
                                in_=t1_all4[pr][64 * k:64 * k + 16, :])
                            srcS2 = bass.AP(tensor=eshd[pr][:].tensor,
                                            offset=16640 * k,
                                            ap=[[1040, 16], [1, 1024]])
                            nc.sync.dma_start(
                                out=esh4[pr][64 * k:64 * k + 16, :],
                                in_=srcS2)

                def emit_relv(h):
                    """rel-v matmuls for head h, folded into pair_sb."""
                    hh2 = h % 2
                    hp = h // 2
                    pr = hp % 2
                    po2 = 64 * hh2
                    for cc in range(2):
                        pr2 = ps_m.tile([128, 512], F32, name="pr2", tag="trp")
                        nc.tensor.matmul(
                            pr2[po2:po2 + 64, :],
                            dlv_sb[po2:po2 + 16, :],
                            esh4[pr][po2:po2 + 16, 512 * cc:512 * (cc + 1)],
                            start=True, stop=True, skip_group_check=True,
                        )
                        nc.vector.tensor_add(
                            pair_sb[hp][po2:po2 + 64, 512 * cc:512 * (cc + 1)],
                            pair_sb[hp][po2:po2 + 64, 512 * cc:512 * (cc + 1)],
                            pr2[po2:po2 + 64, :])
                    if hh2 == 1:
                        with nc.allow_low_precision(reason="bf16 softmax norm"):
                            nc.vector.reciprocal(recd[hp][:], recd[hp][:])
                        for cc in range(2):
                            psb = ps_m.tile([128, 512], F32, name="psb",
                                            tag="trp")
                            nc.tensor.matmul(
                                psb[:], selm2_sb[:],
                                recd[hp][:, 512 * cc:512 * (cc + 1)],
                                start=True, stop=True)
                            nc.vector.tensor_mul(
                                pair_sb[hp][:, 512 * cc:512 * (cc + 1)],
                                pair_sb[hp][:, 512 * cc:512 * (cc + 1)],
                                psb[:])

                # software-pipelined schedule
                w0 = emit_wload(0)
                w1 = emit_wload(1)
                qk_tiles[0] = emit_qkproj(0, w0)
                qk_tiles[1] = emit_qkproj(1, w1)
                emit_dp(0)
                emit_dp(1)
                emit_s1(0)
                emit_s1(1)
                w_next = emit_wload(2)
                relv_q = []
                for h in range(16):
                    hp, hh = h // 2, h % 2
                    if hh == 1 and hp + 2 < 8:
                        emit_dp(hp + 2)
                    if h + 2 < 16:
                        emit_s1(h + 2)
                    if h >= 2:
                        emit_s3(h - 2)
                        if (h - 2) % 2 == 1:
                            qq = (h - 2) // 2
                            relv_q += [2 * qq, 2 * qq + 1]
                    if relv_q:
                        emit_relv(relv_q.pop(0))
                    if 10 <= h <= 13:
                        for d2 in range(2 * (h - 10), 2 * (h - 10) + 2):
                            nc.sync.dma_start(
                                out=Wp_sb[d2][:],
                                in_=Wp[128 * d2:128 * (d2 + 1), :])
                    emit_s2(h)
                    if hh == 0 and hp + 2 < 8:
                        qk_tiles[hp + 2] = emit_qkproj(hp + 2, w_next)
                        if hp + 3 < 8:
                            w_next = emit_wload(hp + 3)
                emit_s3(14)
                emit_s3(15)
                relv_q += [14, 15]
                while relv_q:
                    emit_relv(relv_q.pop(0))

            # ---- final projection ----
            with (
                tc.tile_pool(name="ps_p", bufs=2, space="PSUM") as ps_p,
                tc.tile_pool(name="outp", bufs=2) as outp,
            ):
                for tt in range(8):
                    ps = ps_p.tile([128, 1024], F32, tag="psp")
                    for fc in range(2):
                        for d in range(8):
                            nc.tensor.matmul(
                                ps[:, 512 * fc:512 * (fc + 1)],
                                pair_sb[d][:, 128 * tt:128 * (tt + 1)],
                                Wp_sb[d][:, 512 * fc:512 * (fc + 1)],
                                start=(d == 0), stop=False,
                            )
                        nc.tensor.matmul(
                            ps[:, 512 * fc:512 * (fc + 1)],
                            ones1[:], bp_sb[:, 512 * fc:512 * (fc + 1)],
                            start=False, stop=True,
                        )
                    ot = outp.tile([128, 1024], F32, tag="ot")
                    nc.vector.tensor_copy(ot[:], ps[:])
                    nc.sync.dma_start(out=OUT[128 * tt:128 * (tt + 1), :],
                                      in_=ot[:])

    nc.compile()
    return nc


def _to_bf16(a):
    import ml_dtypes
    return np.asarray(a, np.float32).astype(ml_dtypes.bfloat16)


def _host_prep(W_attn, b_attn, W_proj, b_proj, lut_k, lut_v):
    scale = 1.0 / math.sqrt(d_k)
    Wq = W_attn[:, :D]
    Wk = W_attn[:, D:2 * D] * scale
    # packed per-pair layout: [hp, p, sec*1024 + d*128 + c]
    Wq4 = Wq.reshape(8, 128, 8, 128).transpose(2, 1, 0, 3).reshape(8, 128, 1024)
    Wk4 = Wk.reshape(8, 128, 8, 128).transpose(2, 1, 0, 3).reshape(8, 128, 1024)
    Wqk_h = np.concatenate([Wq4, Wk4], axis=2).reshape(1024, 2048)
    bq = b_attn[:D]
    bk = b_attn[D:2 * D] * scale
    bqk_h = np.stack([np.concatenate([bq, bk])[128 * ft:128 * (ft + 1)]
                      for ft in range(16)], axis=1).astype(np.float32)
    bvp_h = (b_attn[2 * D:3 * D] + np.tile(lut_v[0], N_H)).reshape(1, D)
    dlut_h = np.stack([(lut_k[16 - u] - lut_k[0]) * scale for u in range(16)],
                      axis=1)
    dlv_h = np.stack([lut_v[16 - u] - lut_v[0] for u in range(16)], axis=0)
    selm2_h = np.zeros((2, 128), np.float32)
    selm2_h[0, 0:64] = 1.0
    selm2_h[1, 64:128] = 1.0
    zb2_h = np.where(np.arange(160)[None, :] < np.arange(128)[:, None],
                     np.float32(MASKVAL), np.float32(0.0)).astype(np.float32)
    return {
        "Wqk": _to_bf16(Wqk_h),
        "Wv": _to_bf16(W_attn[:, 2 * D:3 * D]),
        "Wp": _to_bf16(W_proj),
        "bqk": bqk_h,
        "bvp": _to_bf16(bvp_h),
        "bp": _to_bf16(np.asarray(b_proj).reshape(1, D)),
        "dlut": _to_bf16(dlut_h),
        "dlv": _to_bf16(dlv_h),
        "selm2": _to_bf16(selm2_h),
        "zb2init": _to_bf16(zb2_h),
        "zeros64": _to_bf16(np.zeros((64, 1040), np.float32)),
    }


def kernel(x, W_attn, b_attn, W_proj, b_proj, lut_k, lut_v):
    x = np.asarray(x, np.float32)
    shared = _host_prep(np.asarray(W_attn, np.float32),
                        np.asarray(b_attn, np.float32),
                        np.asarray(W_proj, np.float32),
                        np.asarray(b_proj, np.float32),
                        np.asarray(lut_k, np.float32),
                        np.asarray(lut_v, np.float32))
    if "nc" not in _CACHE:
        _CACHE["nc"] = build_module()
    nc = _CACHE["nc"]
    in_maps = []
    for b in range(N_CORES):
        m = dict(shared)
        m["xT"] = _to_bf16(np.ascontiguousarray(x[b].T))
        in_maps.append(m)
    res = run_bass_kernel_spmd(nc, in_maps, list(range(N_CORES)), trace=TRACE)
    _CACHE["last_result"] = res
    out = np.stack([res.results[b]["OUT"] for b in range(N_CORES)], axis=0)
    return out.astype(np.float32)


# revision 4
# speedup vs baseline: 1.2209x; 1.1902x over previous
import sys

sys.path.insert(0, "/opt/trn_rl_repo")

import math

import numpy as np

import concourse.bass as bass
import concourse.mybir as mybir
import concourse.tile as tile
from concourse import bacc
from concourse.bass_utils import run_bass_kernel_spmd
from concourse.masks import make_identity

F32 = mybir.dt.float32
BF16 = mybir.dt.bfloat16
IDENT = mybir.ActivationFunctionType.Identity
EXPF = mybir.ActivationFunctionType.Exp

B, S, D = 8, 1024, 1024
N_H = 16
REL_K = 16
d_k = D // N_H  # 64
N_CORES = 8
MASKVAL = -1e30

_CACHE = {}
TRACE = False


def build_module():
    nc = bacc.Bacc("TRN2", detect_race_conditions=False, num_swdge_queues=4)

    xT = nc.dram_tensor("xT", [D, S], BF16, kind="ExternalInput")
    Wqk = nc.dram_tensor("Wqk", [D, 2 * D], BF16, kind="ExternalInput")
    Wv = nc.dram_tensor("Wv", [D, D], BF16, kind="ExternalInput")
    Wp = nc.dram_tensor("Wp", [D, D], BF16, kind="ExternalInput")
    bqk = nc.dram_tensor("bqk", [128, 16], F32, kind="ExternalInput")
    bvp = nc.dram_tensor("bvp", [1, D], BF16, kind="ExternalInput")
    bp = nc.dram_tensor("bp", [1, D], BF16, kind="ExternalInput")
    dlut = nc.dram_tensor("dlut", [d_k, 16], BF16, kind="ExternalInput")
    dlv = nc.dram_tensor("dlv", [16, d_k], BF16, kind="ExternalInput")
    selm2 = nc.dram_tensor("selm2", [2, 128], BF16, kind="ExternalInput")
    zb2init = nc.dram_tensor("zb2init", [128, 160], BF16, kind="ExternalInput")
    zeros64 = nc.dram_tensor("zeros64", [64, 1040], BF16, kind="ExternalInput")
    OUT = nc.dram_tensor("OUT", [S, D], F32, kind="ExternalOutput")

    # DRAM scratch
    zb2r = [nc.dram_tensor(f"zb2r_{r}", [8 * 128, 160], BF16) for r in range(4)]
    zdp = [nc.dram_tensor(f"zdp_{r}", [32, 1040], BF16) for r in range(3)]
    ewd = [nc.dram_tensor(f"ewd_{r}", [128, 1152], BF16) for r in range(2)]
    eshd = [nc.dram_tensor(f"eshd_{r}", [32, 1040], BF16) for r in range(2)]

    with tile.TileContext(nc) as tc:
        with (
            tc.tile_pool(name="persist", bufs=1) as pers,
        ):
            # ---- load xT first (longest pole for vproj) ----
            xT_sb = []
            for d in range(8):
                t = pers.tile([128, S], BF16, name="xTl", tag=f"xT{d}")
                eng = nc.sync if d % 2 == 0 else nc.scalar
                eng.dma_start(out=t[:], in_=xT[128 * d:128 * (d + 1), :])
                xT_sb.append(t)

            # ---- constants ----
            identF = pers.tile([128, 128], F32)
            make_identity(nc, identF[:])
            identB = pers.tile([128, 128], BF16)
            nc.vector.tensor_copy(identB[:], identF[:])
            dlut_sb = pers.tile([128, 16], BF16)
            nc.sync.dma_start(out=dlut_sb[0:64, :], in_=dlut[:])
            nc.sync.dma_start(out=dlut_sb[64:128, :], in_=dlut[:])
            dlv_sb = pers.tile([128, d_k], BF16)
            nc.sync.dma_start(out=dlv_sb[0:16, :], in_=dlv[:])
            nc.sync.dma_start(out=dlv_sb[64:80, :], in_=dlv[:])
            selm2_sb = pers.tile([2, 128], BF16)
            nc.sync.dma_start(out=selm2_sb[:], in_=selm2[:])
            bqk_sb = pers.tile([128, 16], F32)
            nc.sync.dma_start(out=bqk_sb[:], in_=bqk[:])
            bvp_sb = pers.tile([1, D], BF16)
            nc.sync.dma_start(out=bvp_sb[:], in_=bvp[:])
            bp_sb = pers.tile([1, D], BF16)
            nc.sync.dma_start(out=bp_sb[:], in_=bp[:])
            ones1 = pers.tile([1, 128], BF16)
            nc.vector.memset(ones1[:], 1.0)
            onescol = pers.tile([128, 16], BF16)
            nc.vector.memset(onescol[:], 1.0)

            # init DRAM scratch: masks + zero regions
            zb2i_sb = pers.tile([128, 160], BF16)
            nc.gpsimd.dma_start(out=zb2i_sb[:], in_=zb2init[:])
            for r in range(4):
                for jt in range(8):
                    dst = bass.AP(tensor=zb2r[r][:].tensor, offset=20480 * jt,
                                  ap=[[160, 128], [1, 160]])
                    nc.gpsimd.dma_start(out=dst, in_=zb2i_sb[:])
            z64 = pers.tile([64, 1040], BF16)
            nc.scalar.dma_start(out=z64[:], in_=zeros64[:])
            for r in range(3):
                nc.scalar.dma_start(out=zdp[r][:], in_=z64[0:32, :])
            for r in range(2):
                nc.scalar.dma_start(out=eshd[r][:], in_=z64[0:32, :])

            # ---- v projection target tiles (filled inside attention) ----
            vhat_sb = [pers.tile([128, 16 * 65], BF16, name=f"vh{jt}", tag=f"vh{jt}")
                       for jt in range(8)]
            Wv_sb = [pers.tile([128, D], BF16, name="wvt", tag=f"wv{d}")
                     for d in range(8)]
            for d in range(8):
                eng = nc.sync if d % 2 == 0 else nc.scalar
                eng.dma_start(out=Wv_sb[d][:], in_=Wv[128 * d:128 * (d + 1), :])

            pair_sb = [pers.tile([128, S], BF16, name=f"pair{hp}", tag=f"pair{hp}")
                       for hp in range(8)]
            recd = [pers.tile([2, S], BF16, name=f"recd{hp}",
                               tag=f"recd{hp}") for hp in range(8)]

            Wp_sb = [pers.tile([128, D], BF16, name="wpt", tag=f"wp{d}")
                     for d in range(8)]

            # rotating SBUF tiles
            ew_all = [pers.tile([128, 8 * 512], BF16, name=f"ewall{r}",
                                tag=f"ewall{r}") for r in range(2)]
            for r in range(2):
                # jt=7 cols [128,144) of its chunk never written by exp
                nc.vector.memset(ew_all[r][:, 512 * 7 + 128:512 * 7 + 144], 0.0)
            band_all = [pers.tile([128, 8 * 160], BF16, name=f"bandall{r}",
                                  tag=f"bandall{r}") for r in range(4)]
            dpSh4 = [pers.tile([128, 1024], BF16, name=f"dpSh{r}", tag=f"dpSh{r}")
                     for r in range(3)]
            dp_all4 = [pers.tile([128, 1024], BF16, name=f"dpall{r}",
                                 tag=f"dpall{r}") for r in range(3)]
            t1_all4 = [pers.tile([128, 1024], BF16, name=f"t1all{r}",
                                 tag=f"t1all{r}") for r in range(2)]
            esh4 = [pers.tile([128, 1024], BF16, name=f"esh{r}", tag=f"esh{r}")
                    for r in range(2)]
            esk_all = [pers.tile([128, 128], BF16, name=f"eskall{r}",
                                 tag=f"eskall{r}") for r in range(3)]
            dpS_all = [pers.tile([128, 128], BF16, name=f"dpSall{r}",
                                 tag=f"dpSall{r}") for r in range(4)]

            # ---- attention ----
            with (
                tc.tile_pool(name="wqk", bufs=2) as wqkp,
                tc.tile_pool(name="qk", bufs=6) as qkp,
                tc.tile_pool(name="tail", bufs=4) as tailp,
                tc.tile_pool(name="outtp", bufs=2) as outtp,
                tc.tile_pool(name="ps_s", bufs=4, space="PSUM") as ps_s,
                tc.tile_pool(name="ps_out", bufs=1, space="PSUM") as ps_out,
                tc.tile_pool(name="ps_m", bufs=2, space="PSUM") as ps_m,
            ):

                def emit_wload(hp):
                    w_all = wqkp.tile([128, 2048], BF16, name="w_all",
                                      tag="wqk")
                    src = bass.AP(tensor=Wqk[:].tensor, offset=hp * 128 * 2048,
                                  ap=[[2048, 128], [1, 2048]])
                    nc.sync.dma_start(out=w_all[:], in_=src)
                    return w_all

                def emit_qkproj(hp, w_all):
                    """Produce qT/kT [128 feats(2 heads), 1024 tokens]."""
                    out_pair = []
                    for sec in range(2):
                        ft = hp if sec == 0 else 8 + hp
                        dstt = qkp.tile([128, S], BF16, tag=f"qk{sec}")
                        for tch in range(2):
                            ps = ps_s.tile([128, 512], F32, tag="pss")
                            for d in range(8):
                                nc.tensor.matmul(
                                    ps[:],
                                    w_all[:, 1024 * sec + 128 * d:
                                          1024 * sec + 128 * (d + 1)],
                                    xT_sb[d][:, 512 * tch:512 * (tch + 1)],
                                    start=(d == 0), stop=(d == 7),
                                )
                            nc.scalar.activation(
                                dstt[:, 512 * tch:512 * (tch + 1)], ps[:],
                                IDENT, bias=bqk_sb[:, ft:ft + 1], scale=1.0)
                        out_pair.append(dstt)
                    return out_pair

                qk_tiles = {}  # hp -> (qT, kT)
                relv_state = {}  # hp -> open pso2 tile

                def emit_dp(hp):
                    """dp matmuls + zdp/dpSh shear bounce for pair hp."""
                    pr = hp % 3
                    qT_p = qk_tiles[hp][0]
                    psA = ps_s.tile([128, 512], F32, name="psA", tag="pss")
                    psB = ps_s.tile([128, 512], F32, name="psB", tag="pss")
                    for k in range(2):
                        pok = 64 * k
                        nc.tensor.matmul(
                            psA[64 * k:64 * k + 16, :],
                            dlut_sb[pok:pok + 64, :],
                            qT_p[pok:pok + 64, 0:512],
                            start=True, stop=True, skip_group_check=True)
                        nc.tensor.matmul(
                            psB[64 * k:64 * k + 16, :],
                            dlut_sb[pok:pok + 64, :],
                            qT_p[pok:pok + 64, 512:1024],
                            start=True, stop=True, skip_group_check=True)
                    for k in range(2):
                        nc.vector.tensor_copy(
                            dp_all4[pr][64 * k:64 * k + 16, 0:512],
                            psA[64 * k:64 * k + 16, :])
                        nc.vector.tensor_copy(
                            dp_all4[pr][64 * k:64 * k + 16, 512:1024],
                            psB[64 * k:64 * k + 16, :])
                    for k in range(2):
                        dstz = bass.AP(tensor=zdp[pr][:].tensor,
                                       offset=16640 * k,
                                       ap=[[1040, 16], [1, 1024]])
                        nc.gpsimd.dma_start(out=dstz,
                                            in_=dp_all4[pr][64 * k:64 * k + 16,
                                                            :])
                        srcSh = bass.AP(tensor=zdp[pr][:].tensor,
                                        offset=16640 * k,
                                        ap=[[1041, 16], [1, 1024]])
                        nc.gpsimd.dma_start(
                            out=dpSh4[pr][64 * k:64 * k + 16, :], in_=srcSh)

                def emit_s1(h):
                    """band pipeline for head h: transposes -> diag write ->
                    band+mask rect read."""
                    hp, hh = h // 2, h % 2
                    pr, hr = hp % 3, h % 4
                    psd = ps_m.tile([128, 512], BF16, name="psd", tag="trp")
                    for jt in range(8):
                        nc.tensor.transpose(
                            psd[:, 16 * jt:16 * (jt + 1)],
                            dpSh4[pr][64 * hh:64 * hh + 16,
                                      128 * jt:128 * (jt + 1)],
                            identB[64 * hh:64 * hh + 16,
                                   64 * hh:64 * hh + 16])
                    nc.vector.tensor_copy(dpS_all[hr][:], psd[:, 0:128])
                    dstW = bass.AP(tensor=zb2r[hr][:].tensor, offset=0,
                                   ap=[[161, 128], [20480, 8], [1, 16]])
                    nc.gpsimd.dma_start(out=dstW, in_=dpS_all[hr][:])
                    srcB = bass.AP(tensor=zb2r[hr][:].tensor, offset=0,
                                   ap=[[160, 128], [20480, 8], [1, 160]])
                    nc.gpsimd.dma_start(out=band_all[hr][:], in_=srcB)

                def emit_s2(h):
                    """scores + band add + exp + attn@v + evict + ew/esk DMAs."""
                    hp, hh = h // 2, h % 2
                    po = 64 * hh
                    hr = h % 4
                    ewr = h % 2
                    qT, kT = qk_tiles[hp]
                    pso = ps_out.tile([65, 1024], F32, name="pso", tag="pso")
                    ew = ew_all[ewr]
                    seg_jobs = []

                    def flush_av(jx):
                        lhs, segs = seg_jobs[jx]
                        for si, (a, b2, mt, off) in enumerate(segs):
                            nc.tensor.matmul(
                                pso[:, a:b2], lhs,
                                mt[:, a + off:b2 + off],
                                start=(jx == 0),
                                stop=(jx == 7 and si == len(segs) - 1),
                                skip_group_check=True)

                    for jt in range(8):
                        j0 = 128 * jt
                        wdiag = min(512, S - j0)
                        win = min(144, wdiag)
                        pss = ps_s.tile([128, 512], F32, name="pss", tag="pss")
                        nc.tensor.matmul(pss[:, 0:wdiag],
                                         kT[po:po + 64, j0:j0 + 128],
                                         qT[po:po + 64, j0:j0 + wdiag],
                                         start=True, stop=False,
                                         skip_group_check=True)
                        nc.tensor.matmul(
                            pss[:, 0:win], identB[:],
                            band_all[hr][:, 160 * jt:160 * jt + win],
                            start=False, stop=True, skip_group_check=True)
                        nc.scalar.activation(
                            ew[:, 512 * jt:512 * jt + wdiag],
                            pss[:, 0:wdiag], EXPF)
                        tail = None
                        if S - j0 > 512:
                            w2 = S - j0 - 512
                            pss2 = ps_s.tile([128, 512], F32, name="pss2",
                                             tag="pss")
                            nc.tensor.matmul(pss2[:, 0:w2],
                                             kT[po:po + 64, j0:j0 + 128],
                                             qT[po:po + 64, j0 + 512:S],
                                             start=True, stop=True)
                            tail = tailp.tile([128, 512], BF16, name="tail",
                                              tag="tail")
                            nc.scalar.activation(tail[:, 0:w2], pss2[:, 0:w2],
                                                 EXPF)
                        lhs = vhat_sb[jt][:, 65 * h:65 * h + 65]
                        segs = []
                        if j0 < 512:
                            segs.append((j0, 512, ew, 512 * jt - j0))
                            if j0 + wdiag > 512:
                                segs.append((512, j0 + wdiag, ew,
                                             512 * jt - j0))
                        else:
                            segs.append((j0, j0 + wdiag, ew, 512 * jt - j0))
                        if tail is not None:
                            segs.append((j0 + 512, S, tail, -(j0 + 512)))
                        seg_jobs.append((lhs, segs))
                        if jt >= 3:
                            flush_av(jt - 3)
                    flush_av(5)
                    flush_av(6)
                    flush_av(7)
                    # evict attn@v + denom
                    outT = outtp.tile([65, 1024], BF16, name="outT", tag="outT")
                    nc.vector.tensor_copy(outT[:], pso[:])
                    nc.sync.dma_start(out=pair_sb[hp][po:po + 64, :],
                                      in_=outT[0:64, :])
                    nc.sync.dma_start(out=recd[hp][hh:hh + 1, :],
                                      in_=outT[64:65, :])
                    # ew band write + esk diag read (SP queue, ordered)
                    dstE = bass.AP(tensor=ewd[ewr][:].tensor, offset=0,
                                   ap=[[1152, 128], [144, 8], [1, 144]])
                    srcE = bass.AP(tensor=ew[:].tensor, offset=ew[:].offset,
                                   ap=[[8 * 512, 128], [512, 8], [1, 144]])
                    nc.scalar.dma_start(out=dstE, in_=srcE)
                    srcK = bass.AP(tensor=ewd[ewr][:].tensor, offset=0,
                                   ap=[[1153, 128], [144, 8], [1, 16]])
                    nc.scalar.dma_start(out=esk_all[h % 3][:], in_=srcK)

                def emit_s3(h):
                    """esk transposes + t1 assembly for head h; at pair end,
                    t1 diag write + esh read."""
                    hp, hh = h // 2, h % 2
                    pr = hp % 2
                    eskT = ps_m.tile([128, 512], BF16, name="eskT", tag="trp")
                    eskT2 = ps_m.tile([128, 512], BF16, name="eskT2", tag="trp")
                    for jt in range(8):
                        dst_ps = eskT if jt < 4 else eskT2
                        nc.tensor.transpose(
                            dst_ps[64 * hh:64 * hh + 16,
                                   128 * (jt % 4):128 * (jt % 4) + 128],
                            esk_all[h % 3][:, 16 * jt:16 * (jt + 1)],
                            identB[:])
                    nc.vector.tensor_copy(
                        t1_all4[pr][64 * hh:64 * hh + 16, 0:512],
                        eskT[64 * hh:64 * hh + 16, :])
                    nc.vector.tensor_copy(
                        t1_all4[pr][64 * hh:64 * hh + 16, 512:1024],
                        eskT2[64 * hh:64 * hh + 16, :])
                    if hh == 1:
                        for k in range(2):
                            dstT = bass.AP(tensor=eshd[pr][:].tensor,
                                           offset=16640 * k,
                                           ap=[[1041, 16], [1, 1024]])
                            nc.sync.dma_start(
                                out=dstT,
                                in_=t1_all4[pr][64 * k:64 * k + 16, :])
                            srcS2 = bass.AP(tensor=eshd[pr][:].tensor,
                                            offset=16640 * k,
                                            ap=[[1040, 16], [1, 1024]])
                            nc.sync.dma_start(
                                out=esh4[pr][64 * k:64 * k + 16, :],
                                in_=srcS2)

                def emit_relv(h):
                    """rel-v matmuls for head h, folded into pair_sb."""
                    hh2 = h % 2
                    hp = h // 2
                    pr = hp % 2
                    po2 = 64 * hh2
                    for cc in range(2):
                        pr2 = ps_m.tile([128, 512], F32, name="pr2", tag="trp")
                        nc.tensor.matmul(
                            pr2[po2:po2 + 64, :],
                            dlv_sb[po2:po2 + 16, :],
                            esh4[pr][po2:po2 + 16, 512 * cc:512 * (cc + 1)],
                            start=True, stop=True, skip_group_check=True,
                        )
                        nc.vector.tensor_add(
                            pair_sb[hp][po2:po2 + 64, 512 * cc:512 * (cc + 1)],
                            pair_sb[hp][po2:po2 + 64, 512 * cc:512 * (cc + 1)],
                            pr2[po2:po2 + 64, :])
                    if hh2 == 1:
                        with nc.allow_low_precision(reason="bf16 softmax norm"):
                            nc.vector.reciprocal(recd[hp][:], recd[hp][:])
                        for cc in range(2):
                            psb = ps_m.tile([128, 512], F32, name="psb",
                                            tag="trp")
                            nc.tensor.matmul(
                                psb[:], selm2_sb[:],
                                recd[hp][:, 512 * cc:512 * (cc + 1)],
                                start=True, stop=True)
                            nc.vector.tensor_mul(
                                pair_sb[hp][:, 512 * cc:512 * (cc + 1)],
                                pair_sb[hp][:, 512 * cc:512 * (cc + 1)],
                                psb[:])

                def emit_vproj(tt):
                    vt = vhat_sb[tt]
                    ones_ap = bass.AP(tensor=vt[:].tensor, offset=64,
                                      ap=[[16 * 65, 128], [65, 16]])
                    nc.vector.tensor_copy(ones_ap, onescol[:])
                    for fc in range(2):
                        ps = ps_s.tile([128, 512], F32, name="psv", tag="pss")
                        for d in range(8):
                            nc.tensor.matmul(
                                ps[:],
                                xT_sb[d][:, 128 * tt:128 * (tt + 1)],
                                Wv_sb[d][:, 512 * fc:512 * (fc + 1)],
                                start=(d == 0), stop=False,
                            )
                        nc.tensor.matmul(
                            ps[:], ones1[:], bvp_sb[:, 512 * fc:512 * (fc + 1)],
                            start=False, stop=True,
                        )
                        srcA = bass.AP(tensor=ps[:].tensor, offset=ps[:].offset,
                                       ap=[[512, 128], [64, 8], [1, 64]])
                        dst = bass.AP(tensor=vt[:].tensor, offset=65 * 8 * fc,
                                      ap=[[16 * 65, 128], [65, 8], [1, 64]])
                        nc.scalar.copy(dst, srcA)

                # software-pipelined schedule
                w0 = emit_wload(0)
                w1 = emit_wload(1)
                qk_tiles[0] = emit_qkproj(0, w0)
                qk_tiles[1] = emit_qkproj(1, w1)
                emit_dp(0)
                emit_dp(1)
                for tt in range(8):
                    emit_vproj(tt)
                emit_s1(0)
                emit_s1(1)
                w_next = emit_wload(2)
                relv_q = []
                for h in range(16):
                    hp, hh = h // 2, h % 2
                    if hh == 1 and hp + 2 < 8:
                        emit_dp(hp + 2)
                    if h + 2 < 16:
                        emit_s1(h + 2)
                    if h >= 2:
                        emit_s3(h - 2)
                        if (h - 2) % 2 == 1:
                            qq = (h - 2) // 2
                            relv_q += [2 * qq, 2 * qq + 1]
                    if relv_q:
                        emit_relv(relv_q.pop(0))
                    if 10 <= h <= 13:
                        for d2 in range(2 * (h - 10), 2 * (h - 10) + 2):
                            nc.sync.dma_start(
                                out=Wp_sb[d2][:],
                                in_=Wp[128 * d2:128 * (d2 + 1), :])
                    emit_s2(h)
                    if hh == 0 and hp + 2 < 8:
                        qk_tiles[hp + 2] = emit_qkproj(hp + 2, w_next)
                        if hp + 3 < 8:
                            w_next = emit_wload(hp + 3)
                emit_s3(14)
                emit_s3(15)
                relv_q += [14, 15]
                while relv_q:
                    emit_relv(relv_q.pop(0))

            # ---- final projection ----
            with (
                tc.tile_pool(name="ps_p", bufs=2, space="PSUM") as ps_p,
                tc.tile_pool(name="outp", bufs=2) as outp,
            ):
                for tt in range(8):
                    ps = ps_p.tile([128, 1024], F32, tag="psp")
                    for fc in range(2):
                        for d in range(8):
                            nc.tensor.matmul(
                                ps[:, 512 * fc:512 * (fc + 1)],
                                pair_sb[d][:, 128 * tt:128 * (tt + 1)],
                                Wp_sb[d][:, 512 * fc:512 * (fc + 1)],
                                start=(d == 0), stop=False,
                            )
                        nc.tensor.matmul(
                            ps[:, 512 * fc:512 * (fc + 1)],
                            ones1[:], bp_sb[:, 512 * fc:512 * (fc + 1)],
                            start=False, stop=True,
                        )
                    ot = outp.tile([128, 1024], F32, tag="ot")
                    nc.vector.tensor_copy(ot[:], ps[:])
                    nc.sync.dma_start(out=OUT[128 * tt:128 * (tt + 1), :],
                                      in_=ot[:])

    nc.compile()
    return nc


def _to_bf16(a):
    import ml_dtypes
    return np.asarray(a, np.float32).astype(ml_dtypes.bfloat16)


def _host_prep(W_attn, b_attn, W_proj, b_proj, lut_k, lut_v):
    scale = 1.0 / math.sqrt(d_k)
    Wq = W_attn[:, :D]
    Wk = W_attn[:, D:2 * D] * scale
    # packed per-pair layout: [hp, p, sec*1024 + d*128 + c]
    Wq4 = Wq.reshape(8, 128, 8, 128).transpose(2, 1, 0, 3).reshape(8, 128, 1024)
    Wk4 = Wk.reshape(8, 128, 8, 128).transpose(2, 1, 0, 3).reshape(8, 128, 1024)
    Wqk_h = np.concatenate([Wq4, Wk4], axis=2).reshape(1024, 2048)
    bq = b_attn[:D]
    bk = b_attn[D:2 * D] * scale
    bqk_h = np.stack([np.concatenate([bq, bk])[128 * ft:128 * (ft + 1)]
                      for ft in range(16)], axis=1).astype(np.float32)
    bvp_h = (b_attn[2 * D:3 * D] + np.tile(lut_v[0], N_H)).reshape(1, D)
    dlut_h = np.stack([(lut_k[16 - u] - lut_k[0]) * scale for u in range(16)],
                      axis=1)
    dlv_h = np.stack([lut_v[16 - u] - lut_v[0] for u in range(16)], axis=0)
    selm2_h = np.zeros((2, 128), np.float32)
    selm2_h[0, 0:64] = 1.0
    selm2_h[1, 64:128] = 1.0
    zb2_h = np.where(np.arange(160)[None, :] < np.arange(128)[:, None],
                     np.float32(MASKVAL), np.float32(0.0)).astype(np.float32)
    return {
        "Wqk": _to_bf16(Wqk_h),
        "Wv": _to_bf16(W_attn[:, 2 * D:3 * D]),
        "Wp": _to_bf16(W_proj),
        "bqk": bqk_h,
        "bvp": _to_bf16(bvp_h),
        "bp": _to_bf16(np.asarray(b_proj).reshape(1, D)),
        "dlut": _to_bf16(dlut_h),
        "dlv": _to_bf16(dlv_h),
        "selm2": _to_bf16(selm2_h),
        "zb2init": _to_bf16(zb2_h),
        "zeros64": _to_bf16(np.zeros((64, 1040), np.float32)),
    }


def kernel(x, W_attn, b_attn, W_proj, b_proj, lut_k, lut_v):
    x = np.asarray(x, np.float32)
    shared = _host_prep(np.asarray(W_attn, np.float32),
                        np.asarray(b_attn, np.float32),
                        np.asarray(W_proj, np.float32),
                        np.asarray(b_proj, np.float32),
                        np.asarray(lut_k, np.float32),
                        np.asarray(lut_v, np.float32))
    if "nc" not in _CACHE:
        _CACHE["nc"] = build_module()
    nc = _CACHE["nc"]
    in_maps = []
    for b in range(N_CORES):
        m = dict(shared)
        m["xT"] = _to_bf16(np.ascontiguousarray(x[b].T))
        in_maps.append(m)
    res = run_bass_kernel_spmd(nc, in_maps, list(range(N_CORES)), trace=TRACE)
    _CACHE["last_result"] = res
    out = np.stack([res.results[b]["OUT"] for b in range(N_CORES)], axis=0)
    return out.astype(np.float32)


# revision 5
# speedup vs baseline: 1.2227x; 1.0015x over previous
import sys

sys.path.insert(0, "/opt/trn_rl_repo")

import math

import numpy as np

import concourse.bass as bass
import concourse.mybir as mybir
import concourse.tile as tile
from concourse import bacc
from concourse.bass_utils import run_bass_kernel_spmd
from concourse.masks import make_identity

F32 = mybir.dt.float32
BF16 = mybir.dt.bfloat16
IDENT = mybir.ActivationFunctionType.Identity
EXPF = mybir.ActivationFunctionType.Exp

B, S, D = 8, 1024, 1024
N_H = 16
REL_K = 16
d_k = D // N_H  # 64
N_CORES = 8
MASKVAL = -1e30

_CACHE = {}
TRACE = False


def build_module():
    nc = bacc.Bacc("TRN2", detect_race_conditions=False, num_swdge_queues=4)

    xT = nc.dram_tensor("xT", [D, S], BF16, kind="ExternalInput")
    Wqk = nc.dram_tensor("Wqk", [D, 2 * D], BF16, kind="ExternalInput")
    Wv = nc.dram_tensor("Wv", [D, D], BF16, kind="ExternalInput")
    Wp = nc.dram_tensor("Wp", [D, D], BF16, kind="ExternalInput")
    bqk = nc.dram_tensor("bqk", [128, 16], F32, kind="ExternalInput")
    bvp = nc.dram_tensor("bvp", [1, D], BF16, kind="ExternalInput")
    bp = nc.dram_tensor("bp", [1, D], BF16, kind="ExternalInput")
    dlut = nc.dram_tensor("dlut", [d_k, 16], BF16, kind="ExternalInput")
    dlv = nc.dram_tensor("dlv", [16, d_k], BF16, kind="ExternalInput")
    selm2 = nc.dram_tensor("selm2", [2, 128], BF16, kind="ExternalInput")
    zb2init = nc.dram_tensor("zb2init", [128, 1280], BF16, kind="ExternalInput")
    zeros64 = nc.dram_tensor("zeros64", [64, 1040], BF16, kind="ExternalInput")
    OUT = nc.dram_tensor("OUT", [S, D], F32, kind="ExternalOutput")

    # DRAM scratch
    zb2r = [nc.dram_tensor(f"zb2r_{r}", [8 * 128, 160], BF16) for r in range(4)]
    zdp = [nc.dram_tensor(f"zdp_{r}", [32, 1040], BF16) for r in range(3)]
    ewd = [nc.dram_tensor(f"ewd_{r}", [128, 1152], BF16) for r in range(2)]
    eshd = [nc.dram_tensor(f"eshd_{r}", [32, 1040], BF16) for r in range(2)]

    with tile.TileContext(nc) as tc:
        with (
            tc.tile_pool(name="persist", bufs=1) as pers,
        ):
            # ---- first-pair qk weights, then xT ----
            w01_sb = [pers.tile([128, 2048], BF16, name="w01", tag=f"w01_{i}")
                      for i in range(2)]
            for i in range(2):
                srcw = bass.AP(tensor=Wqk[:].tensor, offset=i * 128 * 2048,
                               ap=[[2048, 128], [1, 2048]])
                eng = nc.sync if i == 0 else nc.scalar
                eng.dma_start(out=w01_sb[i][:], in_=srcw)
            xT_sb = []
            for d in range(8):
                t = pers.tile([128, S], BF16, name="xTl", tag=f"xT{d}")
                eng = nc.sync if d % 2 == 0 else nc.scalar
                eng.dma_start(out=t[:], in_=xT[128 * d:128 * (d + 1), :])
                xT_sb.append(t)

            # ---- constants ----
            identF = pers.tile([128, 128], F32)
            make_identity(nc, identF[:])
            identB = pers.tile([128, 128], BF16)
            nc.vector.tensor_copy(identB[:], identF[:])
            dlut_sb = pers.tile([128, 16], BF16)
            nc.gpsimd.dma_start(out=dlut_sb[0:64, :], in_=dlut[:])
            nc.gpsimd.dma_start(out=dlut_sb[64:128, :], in_=dlut[:])
            dlv_sb = pers.tile([128, d_k], BF16)
            nc.gpsimd.dma_start(out=dlv_sb[0:16, :], in_=dlv[:])
            nc.gpsimd.dma_start(out=dlv_sb[64:80, :], in_=dlv[:])
            selm2_sb = pers.tile([2, 128], BF16)
            nc.gpsimd.dma_start(out=selm2_sb[:], in_=selm2[:])
            bqk_sb = pers.tile([128, 16], F32)
            nc.sync.dma_start(out=bqk_sb[:], in_=bqk[:])
            bvp_sb = pers.tile([1, D], BF16)
            nc.sync.dma_start(out=bvp_sb[:], in_=bvp[:])
            bp_sb = pers.tile([1, D], BF16)
            nc.sync.dma_start(out=bp_sb[:], in_=bp[:])
            ones1 = pers.tile([1, 128], BF16)
            nc.vector.memset(ones1[:], 1.0)
            onescol = pers.tile([128, 16], BF16)
            nc.vector.memset(onescol[:], 1.0)

            # init DRAM scratch: masks via one HWDGE DMA + DRAM->DRAM copies
            dstM = bass.AP(tensor=zb2r[0][:].tensor, offset=0,
                           ap=[[160, 128], [20480, 8], [1, 160]])
            srcM = bass.AP(tensor=zb2init[:].tensor, offset=0,
                           ap=[[1280, 128], [160, 8], [1, 160]])
            nc.sync.dma_start(out=dstM, in_=srcM)
            for r in range(1, 4):
                nc.sync.dma_start(out=zb2r[r][:], in_=zb2r[0][:])
            z64 = pers.tile([64, 1040], BF16)
            nc.scalar.dma_start(out=z64[:], in_=zeros64[:])
            for r in range(3):
                nc.scalar.dma_start(out=zdp[r][:], in_=z64[0:32, :])
            for r in range(2):
                nc.scalar.dma_start(out=eshd[r][:], in_=z64[0:32, :])

            # ---- v projection target tiles (filled inside attention) ----
            vhat_sb = [pers.tile([128, 16 * 65], BF16, name=f"vh{jt}", tag=f"vh{jt}")
                       for jt in range(8)]
            Wv_sb = [pers.tile([128, D], BF16, name="wvt", tag=f"wv{d}")
                     for d in range(8)]
            for d in range(8):
                eng = nc.sync if d % 2 == 0 else nc.scalar
                eng.dma_start(out=Wv_sb[d][:], in_=Wv[128 * d:128 * (d + 1), :])

            pair_sb = [pers.tile([128, S], BF16, name=f"pair{hp}", tag=f"pair{hp}")
                       for hp in range(8)]
            recd = [pers.tile([2, S], BF16, name=f"recd{hp}",
                               tag=f"recd{hp}") for hp in range(8)]

            Wp_sb = [pers.tile([128, D], BF16, name="wpt", tag=f"wp{d}")
                     for d in range(8)]

            # rotating SBUF tiles
            ew_all = [pers.tile([128, 8 * 512], BF16, name=f"ewall{r}",
                                tag=f"ewall{r}") for r in range(2)]
            for r in range(2):
                # jt=7 cols [128,144) of its chunk never written by exp
                nc.vector.memset(ew_all[r][:, 512 * 7 + 128:512 * 7 + 144], 0.0)
            band_all = [pers.tile([128, 8 * 160], BF16, name=f"bandall{r}",
                                  tag=f"bandall{r}") for r in range(4)]
            dpSh4 = [pers.tile([128, 1024], BF16, name=f"dpSh{r}", tag=f"dpSh{r}")
                     for r in range(3)]
            dp_all4 = [pers.tile([128, 1024], BF16, name=f"dpall{r}",
                                 tag=f"dpall{r}") for r in range(3)]
            t1_all4 = [pers.tile([128, 1024], BF16, name=f"t1all{r}",
                                 tag=f"t1all{r}") for r in range(2)]
            esh4 = [pers.tile([128, 1024], BF16, name=f"esh{r}", tag=f"esh{r}")
                    for r in range(2)]
            esk_all = [pers.tile([128, 128], BF16, name=f"eskall{r}",
                                 tag=f"eskall{r}") for r in range(3)]
            dpS_all = [pers.tile([128, 128], BF16, name=f"dpSall{r}",
                                 tag=f"dpSall{r}") for r in range(4)]

            # ---- attention ----
            with (
                tc.tile_pool(name="wqk", bufs=2) as wqkp,
                tc.tile_pool(name="qk", bufs=6) as qkp,
                tc.tile_pool(name="tail", bufs=4) as tailp,
                tc.tile_pool(name="outtp", bufs=2) as outtp,
                tc.tile_pool(name="ps_s", bufs=4, space="PSUM") as ps_s,
                tc.tile_pool(name="ps_out", bufs=1, space="PSUM") as ps_out,
                tc.tile_pool(name="ps_m", bufs=2, space="PSUM") as ps_m,
            ):

                def emit_wload(hp):
                    w_all = wqkp.tile([128, 2048], BF16, name="w_all",
                                      tag="wqk")
                    src = bass.AP(tensor=Wqk[:].tensor, offset=hp * 128 * 2048,
                                  ap=[[2048, 128], [1, 2048]])
                    nc.sync.dma_start(out=w_all[:], in_=src)
                    return w_all

                def emit_qkproj(hp, w_all):
                    """Produce qT/kT [128 feats(2 heads), 1024 tokens]."""
                    out_pair = []
                    for sec in range(2):
                        ft = hp if sec == 0 else 8 + hp
                        dstt = qkp.tile([128, S], BF16, tag=f"qk{sec}")
                        for tch in range(2):
                            ps = ps_s.tile([128, 512], F32, tag="pss")
                            for d in range(8):
                                nc.tensor.matmul(
                                    ps[:],
                                    w_all[:, 1024 * sec + 128 * d:
                                          1024 * sec + 128 * (d + 1)],
                                    xT_sb[d][:, 512 * tch:512 * (tch + 1)],
                                    start=(d == 0), stop=(d == 7),
                                )
                            nc.vector.tensor_scalar(
                                dstt[:, 512 * tch:512 * (tch + 1)], ps[:],
                                bqk_sb[:, ft:ft + 1], None,
                                mybir.AluOpType.add)
                        out_pair.append(dstt)
                    return out_pair

                qk_tiles = {}  # hp -> (qT, kT)
                relv_state = {}  # hp -> open pso2 tile

                def emit_dp(hp):
                    """dp matmuls + zdp/dpSh shear bounce for pair hp."""
                    pr = hp % 3
                    qT_p = qk_tiles[hp][0]
                    psA = ps_s.tile([128, 512], F32, name="psA", tag="pss")
                    psB = ps_s.tile([128, 512], F32, name="psB", tag="pss")
                    for k in range(2):
                        pok = 64 * k
                        nc.tensor.matmul(
                            psA[64 * k:64 * k + 16, :],
                            dlut_sb[pok:pok + 64, :],
                            qT_p[pok:pok + 64, 0:512],
                            start=True, stop=True, skip_group_check=True)
                        nc.tensor.matmul(
                            psB[64 * k:64 * k + 16, :],
                            dlut_sb[pok:pok + 64, :],
                            qT_p[pok:pok + 64, 512:1024],
                            start=True, stop=True, skip_group_check=True)
                    for k in range(2):
                        nc.vector.tensor_copy(
                            dp_all4[pr][64 * k:64 * k + 16, 0:512],
                            psA[64 * k:64 * k + 16, :])
                        nc.vector.tensor_copy(
                            dp_all4[pr][64 * k:64 * k + 16, 512:1024],
                            psB[64 * k:64 * k + 16, :])
                    for k in range(2):
                        dstz = bass.AP(tensor=zdp[pr][:].tensor,
                                       offset=16640 * k,
                                       ap=[[1040, 16], [1, 1024]])
                        nc.gpsimd.dma_start(out=dstz,
                                            in_=dp_all4[pr][64 * k:64 * k + 16,
                                                            :])
                        srcSh = bass.AP(tensor=zdp[pr][:].tensor,
                                        offset=16640 * k,
                                        ap=[[1041, 16], [1, 1024]])
                        nc.gpsimd.dma_start(
                            out=dpSh4[pr][64 * k:64 * k + 16, :], in_=srcSh)

                def emit_s1(h):
                    """band pipeline for head h: transposes -> diag write ->
                    band+mask rect read."""
                    hp, hh = h // 2, h % 2
                    pr, hr = hp % 3, h % 4
                    psd = ps_m.tile([128, 512], BF16, name="psd", tag="trp")
                    for jt in range(8):
                        nc.tensor.transpose(
                            psd[:, 16 * jt:16 * (jt + 1)],
                            dpSh4[pr][64 * hh:64 * hh + 16,
                                      128 * jt:128 * (jt + 1)],
                            identB[64 * hh:64 * hh + 16,
                                   64 * hh:64 * hh + 16])
                    nc.vector.tensor_copy(dpS_all[hr][:], psd[:, 0:128])
                    dstW = bass.AP(tensor=zb2r[hr][:].tensor, offset=0,
                                   ap=[[161, 128], [20480, 8], [1, 16]])
                    nc.gpsimd.dma_start(out=dstW, in_=dpS_all[hr][:])
                    srcB = bass.AP(tensor=zb2r[hr][:].tensor, offset=0,
                                   ap=[[160, 128], [20480, 8], [1, 160]])
                    nc.gpsimd.dma_start(out=band_all[hr][:], in_=srcB)

                def emit_s2(h):
                    """scores + band add + exp + attn@v + evict + ew/esk DMAs."""
                    hp, hh = h // 2, h % 2
                    po = 64 * hh
                    hr = h % 4
                    ewr = h % 2
                    qT, kT = qk_tiles[hp]
                    pso = ps_out.tile([65, 1024], F32, name="pso", tag="pso")
                    ew = ew_all[ewr]
                    seg_jobs = []

                    def flush_av(jx):
                        lhs, segs = seg_jobs[jx]
                        for si, (a, b2, mt, off) in enumerate(segs):
                            nc.tensor.matmul(
                                pso[:, a:b2], lhs,
                                mt[:, a + off:b2 + off],
                                start=(jx == 0),
                                stop=(jx == 7 and si == len(segs) - 1),
                                skip_group_check=True)

                    for jt in range(8):
                        j0 = 128 * jt
                        wdiag = min(512, S - j0)
                        win = min(144, wdiag)
                        pss = ps_s.tile([128, 512], F32, name="pss", tag="pss")
                        nc.tensor.matmul(pss[:, 0:wdiag],
                                         kT[po:po + 64, j0:j0 + 128],
                                         qT[po:po + 64, j0:j0 + wdiag],
                                         start=True, stop=False,
                                         skip_group_check=True)
                        nc.tensor.matmul(
                            pss[:, 0:win], identB[:],
                            band_all[hr][:, 160 * jt:160 * jt + win],
                            start=False, stop=True, skip_group_check=True)
                        nc.scalar.activation(
                            ew[:, 512 * jt:512 * jt + wdiag],
                            pss[:, 0:wdiag], EXPF)
                        tail = None
                        if S - j0 > 512:
                            w2 = S - j0 - 512
                            pss2 = ps_s.tile([128, 512], F32, name="pss2",
                                             tag="pss")
                            nc.tensor.matmul(pss2[:, 0:w2],
                                             kT[po:po + 64, j0:j0 + 128],
                                             qT[po:po + 64, j0 + 512:S],
                                             start=True, stop=True)
                            tail = tailp.tile([128, 512], BF16, name="tail",
                                              tag="tail")
                            nc.scalar.activation(tail[:, 0:w2], pss2[:, 0:w2],
                                                 EXPF)
                        lhs = vhat_sb[jt][:, 65 * h:65 * h + 65]
                        segs = []
                        if j0 < 512:
                            segs.append((j0, 512, ew, 512 * jt - j0))
                            if j0 + wdiag > 512:
                                segs.append((512, j0 + wdiag, ew,
                                             512 * jt - j0))
                        else:
                            segs.append((j0, j0 + wdiag, ew, 512 * jt - j0))
                        if tail is not None:
                            segs.append((j0 + 512, S, tail, -(j0 + 512)))
                        seg_jobs.append((lhs, segs))
                        if jt >= 3:
                            flush_av(jt - 3)
                    flush_av(5)
                    flush_av(6)
                    flush_av(7)
                    # evict attn@v + denom
                    outT = outtp.tile([65, 1024], BF16, name="outT", tag="outT")
                    nc.vector.tensor_copy(outT[:], pso[:])
                    nc.sync.dma_start(out=pair_sb[hp][po:po + 64, :],
                                      in_=outT[0:64, :])
                    nc.sync.dma_start(out=recd[hp][hh:hh + 1, :],
                                      in_=outT[64:65, :])
                    # ew band write + esk diag read (SP queue, ordered)
                    dstE = bass.AP(tensor=ewd[ewr][:].tensor, offset=0,
                                   ap=[[1152, 128], [144, 8], [1, 144]])
                    srcE = bass.AP(tensor=ew[:].tensor, offset=ew[:].offset,
                                   ap=[[8 * 512, 128], [512, 8], [1, 144]])
                    nc.sync.dma_start(out=dstE, in_=srcE)
                    srcK = bass.AP(tensor=ewd[ewr][:].tensor, offset=0,
                                   ap=[[1153, 128], [144, 8], [1, 16]])
                    nc.sync.dma_start(out=esk_all[h % 3][:], in_=srcK)

                def emit_s3(h):
                    """esk transposes + t1 assembly for head h; at pair end,
                    t1 diag write + esh read."""
                    hp, hh = h // 2, h % 2
                    pr = hp % 2
                    eskT = ps_m.tile([128, 512], BF16, name="eskT", tag="trp")
                    eskT2 = ps_m.tile([128, 512], BF16, name="eskT2", tag="trp")
                    for jt in range(8):
                        dst_ps = eskT if jt < 4 else eskT2
                        nc.tensor.transpose(
                            dst_ps[64 * hh:64 * hh + 16,
                                   128 * (jt % 4):128 * (jt % 4) + 128],
                            esk_all[h % 3][:, 16 * jt:16 * (jt + 1)],
                            identB[:])
                    nc.vector.tensor_copy(
                        t1_all4[pr][64 * hh:64 * hh + 16, 0:512],
                        eskT[64 * hh:64 * hh + 16, :])
                    nc.vector.tensor_copy(
                        t1_all4[pr][64 * hh:64 * hh + 16, 512:1024],
                        eskT2[64 * hh:64 * hh + 16, :])
                    if hh == 1:
                        for k in range(2):
                            dstT = bass.AP(tensor=eshd[pr][:].tensor,
                                           offset=16640 * k,
                                           ap=[[1041, 16], [1, 1024]])
                            nc.sync.dma_start(
                                out=dstT,
                                in_=t1_all4[pr][64 * k:64 * k + 16, :])
                            srcS2 = bass.AP(tensor=eshd[pr][:].tensor,
                                            offset=16640 * k,
                                            ap=[[1040, 16], [1, 1024]])
                            nc.sync.dma_start(
                                out=esh4[pr][64 * k:64 * k + 16, :],
                                in_=srcS2)

                def emit_relv(h):
                    """rel-v matmuls for head h, folded into pair_sb."""
                    hh2 = h % 2
                    hp = h // 2
                    pr = hp % 2
                    po2 = 64 * hh2
                    for cc in range(2):
                        pr2 = ps_m.tile([128, 512], F32, name="pr2", tag="trp")
                        nc.tensor.matmul(
                            pr2[po2:po2 + 64, :],
                            dlv_sb[po2:po2 + 16, :],
                            esh4[pr][po2:po2 + 16, 512 * cc:512 * (cc + 1)],
                            start=True, stop=True, skip_group_check=True,
                        )
                        nc.vector.tensor_add(
                            pair_sb[hp][po2:po2 + 64, 512 * cc:512 * (cc + 1)],
                            pair_sb[hp][po2:po2 + 64, 512 * cc:512 * (cc + 1)],
                            pr2[po2:po2 + 64, :])
                    if hh2 == 1:
                        with nc.allow_low_precision(reason="bf16 softmax norm"):
                            nc.vector.reciprocal(recd[hp][:], recd[hp][:])
                        for cc in range(2):
                            psb = ps_m.tile([128, 512], F32, name="psb",
                                            tag="trp")
                            nc.tensor.matmul(
                                psb[:], selm2_sb[:],
                                recd[hp][:, 512 * cc:512 * (cc + 1)],
                                start=True, stop=True)
                            nc.vector.tensor_mul(
                                pair_sb[hp][:, 512 * cc:512 * (cc + 1)],
                                pair_sb[hp][:, 512 * cc:512 * (cc + 1)],
                                psb[:])

                def emit_vproj(tt):
                    vt = vhat_sb[tt]
                    ones_ap = bass.AP(tensor=vt[:].tensor, offset=64,
                                      ap=[[16 * 65, 128], [65, 16]])
                    nc.vector.tensor_copy(ones_ap, onescol[:])
                    for fc in range(2):
                        ps = ps_s.tile([128, 512], F32, name="psv", tag="pss")
                        for d in range(8):
                            nc.tensor.matmul(
                                ps[:],
                                xT_sb[d][:, 128 * tt:128 * (tt + 1)],
                                Wv_sb[d][:, 512 * fc:512 * (fc + 1)],
                                start=(d == 0), stop=False,
                            )
                        nc.tensor.matmul(
                            ps[:], ones1[:], bvp_sb[:, 512 * fc:512 * (fc + 1)],
                            start=False, stop=True,
                        )
                        srcA = bass.AP(tensor=ps[:].tensor, offset=ps[:].offset,
                                       ap=[[512, 128], [64, 8], [1, 64]])
                        dst = bass.AP(tensor=vt[:].tensor, offset=65 * 8 * fc,
                                      ap=[[16 * 65, 128], [65, 8], [1, 64]])
                        nc.scalar.copy(dst, srcA)

                # software-pipelined schedule
                qk_tiles[0] = emit_qkproj(0, w01_sb[0])
                qk_tiles[1] = emit_qkproj(1, w01_sb[1])
                emit_dp(0)
                emit_dp(1)
                for tt in range(8):
                    emit_vproj(tt)
                emit_s1(0)
                emit_s1(1)
                w_next = emit_wload(2)
                relv_q = []
                for h in range(16):
                    hp, hh = h // 2, h % 2
                    if hh == 1 and hp + 2 < 8:
                        emit_dp(hp + 2)
                    if h + 2 < 16:
                        emit_s1(h + 2)
                    if h >= 2:
                        emit_s3(h - 2)
                        if (h - 2) % 2 == 1:
                            qq = (h - 2) // 2
                            relv_q += [2 * qq, 2 * qq + 1]
                    if relv_q:
                        emit_relv(relv_q.pop(0))
                    if 10 <= h <= 13:
                        for d2 in range(2 * (h - 10), 2 * (h - 10) + 2):
                            nc.sync.dma_start(
                                out=Wp_sb[d2][:],
                                in_=Wp[128 * d2:128 * (d2 + 1), :])
                    emit_s2(h)
                    if hh == 0 and hp + 2 < 8:
                        qk_tiles[hp + 2] = emit_qkproj(hp + 2, w_next)
                        if hp + 3 < 8:
                            w_next = emit_wload(hp + 3)
                emit_s3(14)
                emit_s3(15)
                relv_q += [14, 15]
                while relv_q:
                    emit_relv(relv_q.pop(0))

            # ---- final projection ----
            with (
                tc.tile_pool(name="ps_p", bufs=2, space="PSUM") as ps_p,
                tc.tile_pool(name="outp", bufs=2) as outp,
            ):
                for tt in range(8):
                    ps = ps_p.tile([128, 1024], F32, tag="psp")
                    for fc in range(2):
                        for d in range(8):
                            nc.tensor.matmul(
                                ps[:, 512 * fc:512 * (fc + 1)],
                                pair_sb[d][:, 128 * tt:128 * (tt + 1)],
                                Wp_sb[d][:, 512 * fc:512 * (fc + 1)],
                                start=(d == 0), stop=False,
                            )
                        nc.tensor.matmul(
                            ps[:, 512 * fc:512 * (fc + 1)],
                            ones1[:], bp_sb[:, 512 * fc:512 * (fc + 1)],
                            start=False, stop=True,
                        )
                    ot = outp.tile([128, 1024], F32, tag="ot")
                    nc.vector.tensor_copy(ot[:], ps[:])
                    nc.sync.dma_start(out=OUT[128 * tt:128 * (tt + 1), :],
                                      in_=ot[:])

    nc.compile()
    return nc


def _to_bf16(a):
    import ml_dtypes
    return np.asarray(a, np.float32).astype(ml_dtypes.bfloat16)


def _host_prep(W_attn, b_attn, W_proj, b_proj, lut_k, lut_v):
    scale = 1.0 / math.sqrt(d_k)
    Wq = W_attn[:, :D]
    Wk = W_attn[:, D:2 * D] * scale
    # packed per-pair layout: [hp, p, sec*1024 + d*128 + c]
    Wq4 = Wq.reshape(8, 128, 8, 128).transpose(2, 1, 0, 3).reshape(8, 128, 1024)
    Wk4 = Wk.reshape(8, 128, 8, 128).transpose(2, 1, 0, 3).reshape(8, 128, 1024)
    Wqk_h = np.concatenate([Wq4, Wk4], axis=2).reshape(1024, 2048)
    bq = b_attn[:D]
    bk = b_attn[D:2 * D] * scale
    bqk_h = np.stack([np.concatenate([bq, bk])[128 * ft:128 * (ft + 1)]
                      for ft in range(16)], axis=1).astype(np.float32)
    bvp_h = (b_attn[2 * D:3 * D] + np.tile(lut_v[0], N_H)).reshape(1, D)
    dlut_h = np.stack([(lut_k[16 - u] - lut_k[0]) * scale for u in range(16)],
                      axis=1)
    dlv_h = np.stack([lut_v[16 - u] - lut_v[0] for u in range(16)], axis=0)
    selm2_h = np.zeros((2, 128), np.float32)
    selm2_h[0, 0:64] = 1.0
    selm2_h[1, 64:128] = 1.0
    zb2_h = np.where(np.arange(160)[None, :] < np.arange(128)[:, None],
                     np.float32(MASKVAL), np.float32(0.0)).astype(np.float32)
    zb2_h = np.tile(zb2_h, (1, 8))
    return {
        "Wqk": _to_bf16(Wqk_h),
        "Wv": _to_bf16(W_attn[:, 2 * D:3 * D]),
        "Wp": _to_bf16(W_proj),
        "bqk": bqk_h,
        "bvp": _to_bf16(bvp_h),
        "bp": _to_bf16(np.asarray(b_proj).reshape(1, D)),
        "dlut": _to_bf16(dlut_h),
        "dlv": _to_bf16(dlv_h),
        "selm2": _to_bf16(selm2_h),
        "zb2init": _to_bf16(zb2_h),
        "zeros64": _to_bf16(np.zeros((64, 1040), np.float32)),
    }


def kernel(x, W_attn, b_attn, W_proj, b_proj, lut_k, lut_v):
    x = np.asarray(x, np.float32)
    shared = _host_prep(np.asarray(W_attn, np.float32),
                        np.asarray(b_attn, np.float32),
                        np.asarray(W_proj, np.float32),
                        np.asarray(b_proj, np.float32),
                        np.asarray(lut_k, np.float32),
                        np.asarray(lut_v, np.float32))
    if "nc" not in _CACHE:
        _CACHE["nc"] = build_module()
    nc = _CACHE["nc"]
    in_maps = []
    for b in range(N_CORES):
        m = dict(shared)
        m["xT"] = _to_bf16(np.ascontiguousarray(x[b].T))
        in_maps.append(m)
    res = run_bass_kernel_spmd(nc, in_maps, list(range(N_CORES)), trace=TRACE)
    _CACHE["last_result"] = res
    out = np.stack([res.results[b]["OUT"] for b in range(N_CORES)], axis=0)
    return out.astype(np.float32)
